# revision 10
# baseline (speedup 1.0000x reference)
"""Trainium2 Bass kernel for nn_Commnet (gnn_message_passing).

kernel(**inputs) takes FULL unsharded numpy inputs, returns (logp [4096,16],)
computed across 8 NeuronCores (SPMD single program; per-core structure is
carried entirely in input tensors).

Sharding: batches bin-packed into 32 sub-shards (4/core); each sub-shard =
10 batch-groups of 16 batch slots padded to exactly 512 agent slots, so every
512-agent matmul block has a static 16-batch selector window -> the program
is identical on all cores.

EmbeddingBag(mean): table cast to bf16 pre-scaled by 1/8 on host (exact);
the host pre-permutes the per-agent word rows into dim-major slabs
slabT[t][dim, agent*8+word] (a pure row gather/layout, no arithmetic) so the
device streams them with large contiguous HWDGE DMAs at line rate; the
bag-sum is a DVE group-reduce(8) along the free dim, directly producing the
dim-major agent embeddings (no PE transposes, no SWDGE descriptor storm).

Middle layers: emb' = relu(W_l@emb - W_r@(emb*recip) + R'@sel) where
R' = W_r@m + b (x) (len-0.99999) folds expansion+bias; sel is a banded
one-hot*recip selector (1 extra k-tile per block). m = segment sum via DVE
group-sum(8) -> PE transpose -> banded matmuls into disjoint PSUM windows.
Padded agents stay exactly 0 through all layers.
"""

from contextlib import ExitStack

import numpy as np
import ml_dtypes

import concourse.bass as bass
import concourse.bacc as bacc
import concourse.tile as tile
from concourse import mybir
from concourse.masks import make_identity

N_WORDS = 32000
EDIM = 256
N_AGENTS = 131072
BAG = 8
N_BATCHES = 4096
N_ACTIONS = 16
NLAYERS = 3

N_CORES = 8
NSUB = 4                  # sub-shards per core
NSHARD = N_CORES * NSUB   # 32
NBG = 10                  # batch-groups per sub-shard
BG_BATCHES = 16
BG_AGENTS = 512
B_S = NBG * BG_BATCHES    # 160
A_S = NBG * BG_AGENTS     # 5120
G = 8
NG = A_S // G             # 640
NCHUNK = NG // 128        # 5
W2 = 2 * BG_BATCHES       # 32
P = 128
ZPAD_ROW = N_WORDS
DT = mybir.dt
AF = mybir.ActivationFunctionType
OP = mybir.AluOpType

_PROGRAM_CACHE = {}
_DEBUG = False


# ================================================================ host prep
def _pack_batches(counts):
    padded = ((counts + G - 1) // G) * G
    ngroups = NSHARD * NBG
    cap = np.full(ngroups, BG_AGENTS, dtype=np.int64)
    slots = np.full(ngroups, BG_BATCHES, dtype=np.int64)
    members = [[] for _ in range(ngroups)]
    for b in np.argsort(-padded, kind="stable"):
        ok = (cap >= padded[b]) & (slots > 0)
        if not ok.any():
            raise RuntimeError("bin packing failed")
        g = int(np.argmax(np.where(ok, cap, -1)))
        members[g].append(int(b))
        cap[g] -= padded[b]
        slots[g] -= 1
    return members, padded


def _build_host_inputs(x, batch_idx, batch_len, emb_table, W0, b0, W1, b1,
                       Wh, bh):
    bf16 = ml_dtypes.bfloat16
    x = np.asarray(x, dtype=np.int64)
    batch_idx = np.asarray(batch_idx, dtype=np.int64)
    batch_len64 = np.asarray(batch_len, dtype=np.float64)

    counts = np.bincount(batch_idx, minlength=N_BATCHES).astype(np.int64)
    starts = np.concatenate([[0], np.cumsum(counts)[:-1]])
    members, padded = _pack_batches(counts)

    # table rows pre-scaled by 1/8 (exact), bf16, with a zero pad row;
    # uint16 view for fast host-side take/transpose.
    tblu = np.concatenate(
        [np.asarray(emb_table, np.float32) / 8.0,
         np.zeros((1, EDIM), np.float32)], 0).astype(bf16).view(np.uint16)
    # dim-major pre-gathered slabs: slab_all[sh, t, p, a*BAG+k] =
    # table[x[a*BAG+k], t*128+p] / 8 (zeros for padded agent slots).
    slab_all = np.empty((NSHARD, 2, P, A_S * BAG), dtype=np.uint16)
    xsel_all = np.zeros((NSHARD, P, A_S), dtype=bf16)
    recip_all = np.zeros((NSHARD, P, A_S), dtype=bf16)
    s2_all = np.zeros((NSHARD, P, NCHUNK, W2), dtype=bf16)
    lenm1_all = np.zeros((NSHARD, 1, B_S), dtype=bf16)
    gcnt_all = np.full((NSHARD, 1, NBG // 2), BG_AGENTS * 2, dtype=np.int32)
    out_map = np.full((NSHARD, B_S), -1, dtype=np.int64)
    dbg_slots = []
    recip_f = (1.0 / (batch_len64 - 0.99999)).astype(np.float32)

    for sh in range(NSHARD):
        idx_flat = np.full((BAG, A_S), ZPAD_ROW, dtype=np.int64)
        a_of_slot = np.full(A_S, -1, dtype=np.int64)
        b_of_slot = np.full(A_S, -1, dtype=np.int64)
        grp_content = np.zeros(NBG, np.int64)
        for bg in range(NBG):
            pos = bg * BG_AGENTS
            for sl, b in enumerate(members[sh * NBG + bg]):
                lb = bg * BG_BATCHES + sl
                out_map[sh, lb] = b
                lenm1_all[sh, 0, lb] = np.float32(batch_len64[b] - 0.99999)
                n = int(counts[b])
                a_of_slot[pos:pos + n] = np.arange(starts[b], starts[b] + n)
                b_of_slot[pos:pos + n] = lb
                pos += int(padded[b])
            grp_content[bg] = pos - bg * BG_AGENTS
        slots = np.nonzero(a_of_slot >= 0)[0]
        ags = a_of_slot[slots]
        for k in range(BAG):
            idx_flat[k, slots] = x[ags * BAG + k]

        # pre-gather + transpose to dim-major: [256, A_S*BAG]
        gath = tblu[idx_flat.T.ravel()]            # [A_S*BAG, 256]
        slab_all[sh].reshape(EDIM, -1)[:] = gath.T

        lb_real = b_of_slot[slots]
        rec = recip_f[out_map[sh, lb_real]]
        recip_row = np.zeros(A_S, np.float32)
        recip_row[slots] = rec
        recip_all[sh] = np.broadcast_to(recip_row.astype(bf16), (P, A_S))

        j_of_slot = slots // BG_AGENTS
        w0_al = np.where(j_of_slot >= 8, 128, (j_of_slot // 4) * 64)
        r = lb_real - w0_al
        assert (r >= 0).all() and (r < P).all()
        xs = np.zeros((P, A_S), np.float32)
        xs[r, slots] = rec
        xsel_all[sh] = xs.astype(bf16)

        dbg_slots.append((a_of_slot.copy(), b_of_slot.copy()))
        g_b = b_of_slot[::G]
        for c in range(NCHUNK):
            gl = np.arange(P)
            gb = g_b[c * P + gl]
            v = gb >= 0
            w = gb[v] - W2 * c
            assert (w >= 0).all() and (w < W2).all()
            s2_all[sh][gl[v], c, w] = 1.0

    W0 = np.asarray(W0, np.float32)
    W1 = np.asarray(W1, np.float32)
    wl = np.stack([W0[:, :EDIM].T, W1[:, :EDIM].T])   # [layer, 256k, 256d]
    wr = np.stack([W0[:, EDIM:].T, W1[:, EDIM:].T])

    def tiles(w):  # [2,256,256] -> [128, 2(layer), 2(kt), 2(dt), 128]
        t = w.reshape(2, 2, P, 2, P).transpose(2, 0, 1, 3, 4)
        return np.ascontiguousarray(t).astype(bf16)

    host = {
        "wlT": tiles(wl),
        "wrTn": tiles(-wr),
        "wrT": tiles(wr),
        "bias": np.ascontiguousarray(
            np.stack([np.asarray(b0, np.float32), np.asarray(b1, np.float32)])
            .reshape(2, 2, P).transpose(1, 0, 2)[None]  # wrong axis order?
        ),
        "whT": np.ascontiguousarray(
            np.asarray(Wh, np.float32).T.reshape(2, P, N_ACTIONS)
            .transpose(1, 0, 2)).astype(bf16),
        "bh": np.asarray(bh, np.float32).reshape(1, N_ACTIONS).astype(bf16),
        "ones_b": np.ones((1, B_S), bf16),
    }
    # bias layout: [1, 2(layer), 2(dt), 128]
    bias = np.stack([np.asarray(b0, np.float32),
                     np.asarray(b1, np.float32)]).reshape(2, 2, P)
    host["bias"] = bias[None].astype(bf16)

    slab_bf = slab_all.view(bf16)
    per_core = []
    for core in range(N_CORES):
        s0 = core * NSUB
        m = dict(host)
        m["slabT"] = slab_bf[s0:s0 + NSUB]
        m["xsel"] = xsel_all[s0:s0 + NSUB]
        m["recipb"] = recip_all[s0:s0 + NSUB]
        m["s2"] = s2_all[s0:s0 + NSUB]
        m["lenm1"] = lenm1_all[s0:s0 + NSUB]
        per_core.append(m)
    return per_core, out_map, dbg_slots


# ============================================================ device program
def _build_program():
    nc = bacc.Bacc("TRN2")
    bf, f32, i32 = DT.bfloat16, DT.float32, DT.int32

    slab_d = nc.dram_tensor("slabT", [NSUB, 2, P, A_S * BAG], bf,
                            kind="ExternalInput")
    xsel_d = nc.dram_tensor("xsel", [NSUB, P, A_S], bf, kind="ExternalInput")
    recip_d = nc.dram_tensor("recipb", [NSUB, P, A_S], bf,
                             kind="ExternalInput")
    s2_d = nc.dram_tensor("s2", [NSUB, P, NCHUNK, W2], bf,
                          kind="ExternalInput")
    lenm1_d = nc.dram_tensor("lenm1", [NSUB, 1, B_S], bf,
                             kind="ExternalInput")
    wlT_d = nc.dram_tensor("wlT", [P, 2, 2, 2, P], bf, kind="ExternalInput")
    wrTn_d = nc.dram_tensor("wrTn", [P, 2, 2, 2, P], bf, kind="ExternalInput")
    wrT_d = nc.dram_tensor("wrT", [P, 2, 2, 2, P], bf, kind="ExternalInput")
    bias_d = nc.dram_tensor("bias", [1, 2, 2, P], bf, kind="ExternalInput")
    whT_d = nc.dram_tensor("whT", [P, 2, N_ACTIONS], bf,
                           kind="ExternalInput")
    bh_d = nc.dram_tensor("bh", [1, N_ACTIONS], bf, kind="ExternalInput")
    ones_d = nc.dram_tensor("ones_b", [1, B_S], bf, kind="ExternalInput")
    out_d = nc.dram_tensor("out", [NSUB, B_S, N_ACTIONS], f32,
                           kind="ExternalOutput")
    if _DEBUG:
        dbg_emb0 = nc.dram_tensor("dbg_emb0", [2, P, A_S], bf,
                                  kind="ExternalOutput")
        dbg_emb1 = nc.dram_tensor("dbg_emb1", [2, P, A_S], bf,
                                  kind="ExternalOutput")
        dbg_mT = nc.dram_tensor("dbg_mT", [P, 512], f32,
                                kind="ExternalOutput")
        dbg_x2 = nc.dram_tensor("dbg_x2", [2, P, A_S], bf,
                                kind="ExternalOutput")
        dbg_r = nc.dram_tensor("dbg_r", [P, 2 * B_S], f32,
                               kind="ExternalOutput")
        dbg_h = nc.dram_tensor("dbg_h", [N_ACTIONS, B_S], f32,
                               kind="ExternalOutput")

    with tile.TileContext(nc) as tc, ExitStack() as ctx:
        consts = ctx.enter_context(tc.tile_pool(name="consts", bufs=1))
        wpool = ctx.enter_context(tc.tile_pool(name="wpool", bufs=1))
        gpool = ctx.enter_context(tc.tile_pool(name="gath", bufs=2))
        epool = ctx.enter_context(tc.tile_pool(name="emb", bufs=2))
        e1pool = ctx.enter_context(tc.tile_pool(name="emb1", bufs=1))
        xpool = ctx.enter_context(tc.tile_pool(name="x2p", bufs=1))
        spool = ctx.enter_context(tc.tile_pool(name="small", bufs=2))
        main_ps = ctx.enter_context(
            tc.tile_pool(name="mps", bufs=4, space="PSUM"))
        tp_ps = ctx.enter_context(
            tc.tile_pool(name="tps", bufs=2, space="PSUM"))
        sm_ps = ctx.enter_context(
            tc.tile_pool(name="sps", bufs=2, space="PSUM"))

        ident = consts.tile([P, P], f32, tag="ident", name="ident")
        make_identity(nc, ident[:])

        wlT = wpool.tile([P, 2, 2, 2, P], bf, tag="wlT", name="wlT")
        nc.sync.dma_start(wlT[:], wlT_d[:])
        wrTn = wpool.tile([P, 2, 2, 2, P], bf, tag="wrTn", name="wrTn")
        nc.sync.dma_start(wrTn[:], wrTn_d[:])
        wrT = wpool.tile([P, 2, 2, 2, P], bf, tag="wrT", name="wrT")
        nc.sync.dma_start(wrT[:], wrT_d[:])
        bias_sb = wpool.tile([1, 2, 2, P], bf, tag="bias", name="bias")
        nc.sync.dma_start(bias_sb[:], bias_d[:])
        whT = wpool.tile([P, 2, N_ACTIONS], bf, tag="whT", name="whT")
        nc.sync.dma_start(whT[:], whT_d[:])
        bh_sb = wpool.tile([1, N_ACTIONS], bf, tag="bh", name="bh")
        nc.sync.dma_start(bh_sb[:], bh_d[:])
        ones_sb = wpool.tile([1, B_S], bf, tag="ones", name="ones")
        nc.sync.dma_start(ones_sb[:], ones_d[:])

        def tpack(src_aps, dst_ap):
            """PE-transpose [p<=128, w<=128] fp32 APs into one psum bank,
            then one ACT copy (w/ cast) into dst_ap (columns concatenated).
            Each src must have 128 partitions."""
            ps = tp_ps.tile([P, 512], f32, tag="tpack", name="tpack")
            col = 0
            for a in src_aps:
                w = a.shape[-1]
                nc.tensor.transpose(ps[:, col:col + w], a, ident[:])
                col += w
            nc.scalar.activation(dst_ap, ps[:, :col], AF.Copy)

        for s in range(NSUB):
            # ---------------- Phase A: stream pre-gathered slabs + bag-sum
            xsel = gpool.tile([P, A_S], bf, tag="xsel", name="xsel", bufs=1)
            nc.sync.dma_start(xsel[:], xsel_d[s])
            recipb = gpool.tile([P, A_S], bf, tag="recipb", name="recipb", bufs=1)
            nc.sync.dma_start(recipb[:], recip_d[s])
            s2_sb = gpool.tile([P, NCHUNK, W2], bf, tag="s2", name="s2")
            nc.sync.dma_start(s2_sb[:], s2_d[s])
            lenm1 = gpool.tile([1, B_S], bf, tag="lenm1", name="lenm1")
            nc.sync.dma_start(lenm1[:], lenm1_d[s])

            emb = [None] * NLAYERS
            emb[0] = [epool.tile([P, A_S], bf, tag=f"emb0_{t}", name=f"emb0_{t}")
                      for t in range(2)]
            emb[1] = [e1pool.tile([P, A_S], bf, tag=f"emb1_{t}", name=f"emb1_{t}")
                      for t in range(2)]
            emb[2] = [epool.tile([P, A_S], bf, tag=f"emb0_{t}", name=f"emb0_{t}")
                      for t in range(2)]

            for j in range(NBG):
                slab = gpool.tile([P, 2, BG_AGENTS * BAG], bf, tag="slab",
                                  name="slab", bufs=2)
                js = slice(j * BG_AGENTS * BAG, (j + 1) * BG_AGENTS * BAG)
                for t in range(2):
                    nc.sync.dma_start(slab[:, t, :], slab_d[s, t, :, js])
                with nc.allow_low_precision(reason="bag-sum of 8 bf16 rows"):
                    for t in range(2):
                        nc.vector.tensor_reduce(
                            emb[0][t][:, j * BG_AGENTS:(j + 1) * BG_AGENTS],
                            slab[:, t, :].rearrange("p (g e) -> p g e",
                                                    e=BAG),
                            axis=mybir.AxisListType.X, op=OP.add)

            if _DEBUG and s == 0:
                for t in range(2):
                    nc.sync.dma_start(dbg_emb0[t], emb[0][t][:])

            # ---------------- helpers
            def segsum(src):
                """src = [t0, t1] bf16 [P, A_S] -> m^T psum [P, 512] f32:
                cols 0:256 = batches 0:128, cols 256:512 = batches 128:160
                (on partitions 0:32)."""
                grp = [spool.tile([P, NG], f32, tag=f"grp{t}", name=f"grp{t}",
                                  bufs=1)
                       for t in range(2)]
                for t in range(2):
                    nc.vector.tensor_reduce(
                        grp[t][:],
                        src[t][:].rearrange("p (g e) -> p g e", e=G),
                        axis=mybir.AxisListType.X, op=OP.add)
                gt = spool.tile([P, NCHUNK * EDIM], bf, tag="gt", name="gt")
                for c in range(NCHUNK):
                    tpack([grp[t][:, c * P:(c + 1) * P] for t in range(2)],
                          gt[:, c * EDIM:(c + 1) * EDIM])
                m_ps = sm_ps.tile([P, 512], f32, tag="sps", name="sps")
                for c in range(NCHUNK):
                    r0 = W2 * c if c < 4 else 0
                    dst = (m_ps[r0:r0 + W2, 0:EDIM] if c < 4
                           else m_ps[0:W2, EDIM:2 * EDIM])
                    nc.tensor.matmul(dst, lhsT=s2_sb[:, c, :],
                                     rhs=gt[:, c * EDIM:(c + 1) * EDIM],
                                     start=True, stop=True,
                                     skip_group_check=True,
                                     tile_position=(0, r0))
                return m_ps

            def m_to_sbuf(m_ps):
                mT = spool.tile([P, 512], f32, tag="mT", name="mT")
                nc.scalar.activation(mT[:, 0:EDIM], m_ps[:, 0:EDIM], AF.Copy)
                nc.scalar.activation(mT[0:W2, EDIM:2 * EDIM],
                                     m_ps[0:W2, EDIM:2 * EDIM], AF.Copy)
                return mT

            def m_dimmajor(mT_sb):
                """m^T sbuf -> mdm bf16 [P, 2(dt), B_S] (dim-major m)."""
                ps = sm_ps.tile([P, 512], f32, tag="sps", name="sps")
                for t in range(2):
                    nc.tensor.transpose(ps[:, t * B_S:t * B_S + P],
                                        mT_sb[:, t * P:(t + 1) * P],
                                        ident[:])
                    nc.tensor.transpose(
                        ps[:, t * B_S + P:t * B_S + B_S],
                        mT_sb[0:W2, EDIM + t * P:EDIM + (t + 1) * P],
                        ident[0:W2, 0:W2])
                out = spool.tile([P, 2 * B_S], bf, tag="mdm", name="mdm")
                nc.scalar.activation(out[:], ps[:, 0:2 * B_S], AF.Copy)
                return out

            # ---------------- layers 0, 1
            for i in range(2):
                x2 = [xpool.tile([P, A_S], bf, tag=f"x2_{t}", name=f"x2_{t}")
                      for t in range(2)]
                for t in range(2):
                    nc.vector.tensor_tensor(out=x2[t][:], in0=emb[i][t][:],
                                            in1=recipb[:], op=OP.mult)
                m_ps = segsum(emb[i])
                mT_sb_dbg = m_to_sbuf(m_ps)
                if _DEBUG and s == 0 and i == 0:
                    for t in range(2):
                        nc.sync.dma_start(dbg_x2[t], x2[t][:])
                    nc.sync.dma_start(dbg_mT[:], mT_sb_dbg[:])
                mdm = m_dimmajor(mT_sb_dbg)
                r_ps = sm_ps.tile([P, 512], f32, tag="sps", name="sps")
                for dt in range(2):
                    sl = r_ps[:, dt * B_S:(dt + 1) * B_S]
                    for kt in range(2):
                        nc.tensor.matmul(
                            sl, lhsT=wrT[:, i, kt, dt, :],
                            rhs=mdm[:, kt * B_S:(kt + 1) * B_S],
                            start=(kt == 0), stop=False)
                    nc.tensor.matmul(sl, lhsT=bias_sb[:, i, dt, :],
                                     rhs=lenm1[:], start=False, stop=True)
                r_sb = spool.tile([P, 2 * B_S], f32, tag="r_sb", name="r_sb")
                nc.scalar.activation(r_sb[:], r_ps[:, 0:2 * B_S], AF.Copy)
                if _DEBUG and s == 0 and i == 0:
                    nc.sync.dma_start(dbg_r[:], r_sb[:])
                # R^T at alignments 0 / 64 / 128 -> rt [P, 3, 256] bf16
                rt = spool.tile([P, 3, EDIM], bf, tag="rt", name="rt")
                nc.vector.memset(rt[:], 0.0)
                rt_ps = sm_ps.tile([P, 512], f32, tag="sps", name="sps")
                for dt in range(2):
                    nc.tensor.transpose(
                        rt_ps[:, dt * P:(dt + 1) * P],
                        r_sb[:, dt * B_S:dt * B_S + P], ident[:])
                nc.scalar.activation(rt[:, 0, :], rt_ps[:, 0:EDIM], AF.Copy)
                rt_ps2 = sm_ps.tile([P, 512], f32, tag="sps", name="sps")
                for dt in range(2):
                    nc.tensor.transpose(
                        rt_ps2[0:96, dt * P:(dt + 1) * P],
                        r_sb[:, dt * B_S + 64:dt * B_S + B_S], ident[:])
                    nc.tensor.transpose(
                        rt_ps2[0:W2, EDIM + dt * P:EDIM + dt * P + P],
                        r_sb[:, dt * B_S + P:dt * B_S + B_S], ident[:])
                nc.scalar.activation(rt[0:96, 1, :], rt_ps2[0:96, 0:EDIM],
                                     AF.Copy)
                nc.scalar.activation(rt[0:W2, 2, :],
                                     rt_ps2[0:W2, EDIM:2 * EDIM], AF.Copy)
                # main matmuls
                for j in range(NBG):
                    al = min(j // 4, 2)
                    js = slice(j * BG_AGENTS, (j + 1) * BG_AGENTS)
                    for dt in range(2):
                        ps = main_ps.tile([P, BG_AGENTS], f32, tag="main", name="main")
                        for kt in range(2):
                            nc.tensor.matmul(
                                ps[:], lhsT=wlT[:, i, kt, dt, :],
                                rhs=emb[i][kt][:, js],
                                start=(kt == 0), stop=False)
                        for kt in range(2):
                            nc.tensor.matmul(
                                ps[:], lhsT=wrTn[:, i, kt, dt, :],
                                rhs=x2[kt][:, js], start=False, stop=False)
                        nc.tensor.matmul(
                            ps[:], lhsT=rt[:, al, dt * P:(dt + 1) * P],
                            rhs=xsel[:, js], start=False, stop=True)
                        nc.scalar.activation(emb[i + 1][dt][:, js], ps[:],
                                             AF.Relu)
                if _DEBUG and s == 0 and i == 0:
                    for t in range(2):
                        nc.sync.dma_start(dbg_emb1[t], emb[1][t][:])

            # ---------------- final segsum + head + log_softmax
            m_ps = segsum(emb[2])
            mdm = m_dimmajor(m_to_sbuf(m_ps))
            h_ps = sm_ps.tile([P, 512], f32, tag="sps", name="sps")
            hsl = h_ps[0:N_ACTIONS, 0:B_S]
            for kt in range(2):
                nc.tensor.matmul(hsl, lhsT=whT[:, kt, :],
                                 rhs=mdm[:, kt * B_S:(kt + 1) * B_S],
                                 start=(kt == 0), stop=False)
            nc.tensor.matmul(hsl, lhsT=bh_sb[:], rhs=ones_sb[:],
                             start=False, stop=True)
            h_sb = spool.tile([N_ACTIONS, B_S], f32, tag="h_sb", name="h_sb")
            nc.scalar.activation(h_sb[:], hsl, AF.Copy)
            if _DEBUG and s == 0:
                nc.sync.dma_start(dbg_h[:], h_sb[:])
            lg_ps = sm_ps.tile([P, 512], f32, tag="sps", name="sps")
            nc.tensor.transpose(lg_ps[:, 0:N_ACTIONS], h_sb[:, 0:P],
                                ident[0:N_ACTIONS, 0:N_ACTIONS])
            nc.tensor.transpose(lg_ps[0:W2, N_ACTIONS:2 * N_ACTIONS],
                                h_sb[:, P:B_S],
                                ident[0:N_ACTIONS, 0:N_ACTIONS])
            lg = spool.tile([P, 2 * N_ACTIONS], f32, tag="lg_sb", name="lg_sb")
            nc.scalar.activation(lg[:, 0:N_ACTIONS], lg_ps[:, 0:N_ACTIONS],
                                 AF.Copy)
            nc.scalar.activation(lg[0:W2, N_ACTIONS:2 * N_ACTIONS],
                                 lg_ps[0:W2, N_ACTIONS:2 * N_ACTIONS],
                                 AF.Copy)
            for part in range(2):
                rows = P if part == 0 else B_S - P
                src = lg[0:rows, part * N_ACTIONS:(part + 1) * N_ACTIONS]
                mx = spool.tile([P, 1], f32, tag="mx", name="mx")
                nc.vector.tensor_reduce(mx[0:rows, :], src,
                                        axis=mybir.AxisListType.X,
                                        op=OP.max)
                shv = spool.tile([P, N_ACTIONS], f32, tag="shift", name="shift")
                nc.vector.tensor_tensor(
                    out=shv[0:rows, :], in0=src,
                    in1=mx[0:rows, :].to_broadcast([rows, N_ACTIONS]),
                    op=OP.subtract)
                ex = spool.tile([P, N_ACTIONS], f32, tag="ex", name="ex")
                se = spool.tile([P, 1], f32, tag="se", name="se")
                nc.scalar.activation(ex[0:rows, :], shv[0:rows, :], AF.Exp,
                                     accum_out=se[0:rows, :])
                lse = spool.tile([P, 1], f32, tag="lse", name="lse")
                nc.scalar.activation(lse[0:rows, :], se[0:rows, :], AF.Ln)
                res = spool.tile([P, N_ACTIONS], f32, tag="res", name="res")
                nc.vector.tensor_tensor(
                    out=res[0:rows, :], in0=shv[0:rows, :],
                    in1=lse[0:rows, :].to_broadcast([rows, N_ACTIONS]),
                    op=OP.subtract)
                nc.sync.dma_start(out_d[s, part * P:part * P + rows, :],
                                  res[0:rows, :])
    nc.compile()
    return nc


# ================================================================== kernel
def kernel(**inputs):
    per_core, out_map, _ = _build_host_inputs(
        inputs["x"], inputs["batch_idx"], inputs["batch_len"],
        inputs["emb_table"], inputs["W0"], inputs["b0"], inputs["W1"],
        inputs["b1"], inputs["Wh"], inputs["bh"])

    if "prog" not in _PROGRAM_CACHE:
        _PROGRAM_CACHE["prog"] = _build_program()
    nc = _PROGRAM_CACHE["prog"]

    from concourse.bass_utils import run_bass_kernel_spmd
    res = run_bass_kernel_spmd(nc, per_core, core_ids=list(range(N_CORES)))

    logp = np.zeros((N_BATCHES, N_ACTIONS), np.float32)
    for core in range(N_CORES):
        out = np.asarray(res.results[core]["out"], np.float32)
        for s in range(NSUB):
            sh = core * NSUB + s
            v = out_map[sh] >= 0
            logp[out_map[sh][v]] = out[s][v]
    return (logp,)



# revision 15
# speedup vs baseline: 1.2318x; 1.2318x over previous
"""Trainium2 Bass kernel for nn_Commnet (gnn_message_passing).

kernel(**inputs) takes FULL unsharded numpy inputs, returns (logp [4096,16],)
computed across 8 NeuronCores (SPMD single program; per-core structure is
carried entirely in input tensors).

Sharding: batches bin-packed into 32 sub-shards (4/core); each sub-shard =
10 batch-groups of 16 batch slots padded to exactly 512 agent slots, so every
512-agent matmul block has a static 16-batch selector window -> the program
is identical on all cores.

EmbeddingBag(mean): table cast to bf16 pre-scaled by 1/8 on host (exact);
the host pre-permutes the per-agent word rows into dim-major slabs
slabT[t][dim, agent*8+word] (a pure row gather/layout, no arithmetic) so the
device streams them with large contiguous HWDGE DMAs at line rate; the
bag-sum is a DVE group-reduce(8) along the free dim, directly producing the
dim-major agent embeddings (no PE transposes, no SWDGE descriptor storm).

Middle layers: emb' = relu(W_l@emb - W_r@(emb*recip) + R'@sel) where
R' = W_r@m + b (x) (len-0.99999) folds expansion+bias; sel is a banded
one-hot*recip selector (1 extra k-tile per block). m = segment sum via DVE
group-sum(8) -> PE transpose -> banded matmuls into disjoint PSUM windows.
Padded agents stay exactly 0 through all layers.
"""

from contextlib import ExitStack

import numpy as np
import ml_dtypes

import concourse.bass as bass
import concourse.bacc as bacc
import concourse.tile as tile
from concourse import mybir
from concourse.masks import make_identity

N_WORDS = 32000
EDIM = 256
N_AGENTS = 131072
BAG = 8
N_BATCHES = 4096
N_ACTIONS = 16
NLAYERS = 3

N_CORES = 8
NSUB = 4                  # sub-shards per core
NSHARD = N_CORES * NSUB   # 32
NBG = 10                  # batch-groups per sub-shard
BG_BATCHES = 16
BG_AGENTS = 512
B_S = NBG * BG_BATCHES    # 160
A_S = NBG * BG_AGENTS     # 5120
G = 8
NG = A_S // G             # 640
NCHUNK = NG // 128        # 5
W2 = 2 * BG_BATCHES       # 32
P = 128
ZPAD_ROW = N_WORDS
DT = mybir.dt
AF = mybir.ActivationFunctionType
OP = mybir.AluOpType

_PROGRAM_CACHE = {}
_DEBUG = False


# ================================================================ host prep
def _pack_batches(counts):
    padded = ((counts + G - 1) // G) * G
    ngroups = NSHARD * NBG
    cap = np.full(ngroups, BG_AGENTS, dtype=np.int64)
    slots = np.full(ngroups, BG_BATCHES, dtype=np.int64)
    members = [[] for _ in range(ngroups)]
    for b in np.argsort(-padded, kind="stable"):
        ok = (cap >= padded[b]) & (slots > 0)
        if not ok.any():
            raise RuntimeError("bin packing failed")
        g = int(np.argmax(np.where(ok, cap, -1)))
        members[g].append(int(b))
        cap[g] -= padded[b]
        slots[g] -= 1
    return members, padded


def _build_host_inputs(x, batch_idx, batch_len, emb_table, W0, b0, W1, b1,
                       Wh, bh):
    bf16 = ml_dtypes.bfloat16
    x = np.asarray(x, dtype=np.int64)
    batch_idx = np.asarray(batch_idx, dtype=np.int64)
    batch_len64 = np.asarray(batch_len, dtype=np.float64)

    counts = np.bincount(batch_idx, minlength=N_BATCHES).astype(np.int64)
    starts = np.concatenate([[0], np.cumsum(counts)[:-1]])
    members, padded = _pack_batches(counts)

    # table rows pre-scaled by 1/8 (exact), bf16, with a zero pad row;
    # uint16 view for fast host-side take/transpose.
    tblu = np.concatenate(
        [np.asarray(emb_table, np.float32) / 8.0,
         np.zeros((1, EDIM), np.float32)], 0).astype(bf16).view(np.uint16)
    # dim-major word-plane slabs: slab_all[sh, t, k, p, a] =
    # table[x[a*BAG+k], t*128+p] / 8 (zeros for padded agent slots).
    # The device bag-sums the 8 k-planes via SWDGE accumulate-DMA (CCE ADD).
    slab_all = np.empty((NSHARD, 2, BAG, P, A_S), dtype=np.uint16)
    xsel_all = np.zeros((NSHARD, P, A_S), dtype=bf16)
    recip_all = np.zeros((NSHARD, P, A_S), dtype=bf16)
    s2_all = np.zeros((NSHARD, P, NCHUNK, W2), dtype=bf16)
    lenm1_all = np.zeros((NSHARD, 1, B_S), dtype=bf16)
    gcnt_all = np.full((NSHARD, 1, NBG // 2), BG_AGENTS * 2, dtype=np.int32)
    out_map = np.full((NSHARD, B_S), -1, dtype=np.int64)
    dbg_slots = []
    recip_f = (1.0 / (batch_len64 - 0.99999)).astype(np.float32)

    for sh in range(NSHARD):
        idx_flat = np.full((BAG, A_S), ZPAD_ROW, dtype=np.int64)
        a_of_slot = np.full(A_S, -1, dtype=np.int64)
        b_of_slot = np.full(A_S, -1, dtype=np.int64)
        grp_content = np.zeros(NBG, np.int64)
        for bg in range(NBG):
            pos = bg * BG_AGENTS
            for sl, b in enumerate(members[sh * NBG + bg]):
                lb = bg * BG_BATCHES + sl
                out_map[sh, lb] = b
                lenm1_all[sh, 0, lb] = np.float32(batch_len64[b] - 0.99999)
                n = int(counts[b])
                a_of_slot[pos:pos + n] = np.arange(starts[b], starts[b] + n)
                b_of_slot[pos:pos + n] = lb
                pos += int(padded[b])
            grp_content[bg] = pos - bg * BG_AGENTS
        slots = np.nonzero(a_of_slot >= 0)[0]
        ags = a_of_slot[slots]
        for k in range(BAG):
            idx_flat[k, slots] = x[ags * BAG + k]

        # pre-gather + transpose to dim-major word planes
        for k in range(BAG):
            gath = tblu[idx_flat[k]]               # [A_S, 256]
            slab_all[sh, :, k] = gath.T.reshape(2, P, A_S)

        lb_real = b_of_slot[slots]
        rec = recip_f[out_map[sh, lb_real]]
        recip_row = np.zeros(A_S, np.float32)
        recip_row[slots] = rec
        recip_all[sh] = np.broadcast_to(recip_row.astype(bf16), (P, A_S))

        j_of_slot = slots // BG_AGENTS
        w0_al = np.where(j_of_slot >= 8, 128, (j_of_slot // 4) * 64)
        r = lb_real - w0_al
        assert (r >= 0).all() and (r < P).all()
        xs = np.zeros((P, A_S), np.float32)
        xs[r, slots] = rec
        xsel_all[sh] = xs.astype(bf16)

        dbg_slots.append((a_of_slot.copy(), b_of_slot.copy()))
        g_b = b_of_slot[::G]
        for c in range(NCHUNK):
            gl = np.arange(P)
            gb = g_b[c * P + gl]
            v = gb >= 0
            w = gb[v] - W2 * c
            assert (w >= 0).all() and (w < W2).all()
            s2_all[sh][gl[v], c, w] = 1.0

    W0 = np.asarray(W0, np.float32)
    W1 = np.asarray(W1, np.float32)
    wl = np.stack([W0[:, :EDIM].T, W1[:, :EDIM].T])   # [layer, 256k, 256d]
    wr = np.stack([W0[:, EDIM:].T, W1[:, EDIM:].T])

    def tiles(w):  # [2,256,256] -> [128, 2(layer), 2(kt), 2(dt), 128]
        t = w.reshape(2, 2, P, 2, P).transpose(2, 0, 1, 3, 4)
        return np.ascontiguousarray(t).astype(bf16)

    host = {
        "wlT": tiles(wl),
        "wrTn": tiles(-wr),
        "wrT": tiles(wr),
        "bias": np.ascontiguousarray(
            np.stack([np.asarray(b0, np.float32), np.asarray(b1, np.float32)])
            .reshape(2, 2, P).transpose(1, 0, 2)[None]  # wrong axis order?
        ),
        "whT": np.ascontiguousarray(
            np.asarray(Wh, np.float32).T.reshape(2, P, N_ACTIONS)
            .transpose(1, 0, 2)).astype(bf16),
        "bh": np.asarray(bh, np.float32).reshape(1, N_ACTIONS).astype(bf16),
        "ones_b": np.ones((1, B_S), bf16),
    }
    # bias layout: [1, 2(layer), 2(dt), 128]
    bias = np.stack([np.asarray(b0, np.float32),
                     np.asarray(b1, np.float32)]).reshape(2, 2, P)
    host["bias"] = bias[None].astype(bf16)

    slab_bf = slab_all.view(bf16)
    per_core = []
    for core in range(N_CORES):
        s0 = core * NSUB
        m = dict(host)
        m["slabT"] = slab_bf[s0:s0 + NSUB]
        m["xsel"] = xsel_all[s0:s0 + NSUB]
        m["recipb"] = recip_all[s0:s0 + NSUB]
        m["s2"] = s2_all[s0:s0 + NSUB]
        m["lenm1"] = lenm1_all[s0:s0 + NSUB]
        per_core.append(m)
    return per_core, out_map, dbg_slots


# ============================================================ device program
def _build_program():
    nc = bacc.Bacc("TRN2")
    bf, f32, i32 = DT.bfloat16, DT.float32, DT.int32

    slab_d = nc.dram_tensor("slabT", [NSUB, 2, BAG, P, A_S], bf,
                            kind="ExternalInput")
    xsel_d = nc.dram_tensor("xsel", [NSUB, P, A_S], bf, kind="ExternalInput")
    recip_d = nc.dram_tensor("recipb", [NSUB, P, A_S], bf,
                             kind="ExternalInput")
    s2_d = nc.dram_tensor("s2", [NSUB, P, NCHUNK, W2], bf,
                          kind="ExternalInput")
    lenm1_d = nc.dram_tensor("lenm1", [NSUB, 1, B_S], bf,
                             kind="ExternalInput")
    wlT_d = nc.dram_tensor("wlT", [P, 2, 2, 2, P], bf, kind="ExternalInput")
    wrTn_d = nc.dram_tensor("wrTn", [P, 2, 2, 2, P], bf, kind="ExternalInput")
    wrT_d = nc.dram_tensor("wrT", [P, 2, 2, 2, P], bf, kind="ExternalInput")
    bias_d = nc.dram_tensor("bias", [1, 2, 2, P], bf, kind="ExternalInput")
    whT_d = nc.dram_tensor("whT", [P, 2, N_ACTIONS], bf,
                           kind="ExternalInput")
    bh_d = nc.dram_tensor("bh", [1, N_ACTIONS], bf, kind="ExternalInput")
    ones_d = nc.dram_tensor("ones_b", [1, B_S], bf, kind="ExternalInput")
    out_d = nc.dram_tensor("out", [NSUB, B_S, N_ACTIONS], f32,
                           kind="ExternalOutput")
    if _DEBUG:
        dbg_emb0 = nc.dram_tensor("dbg_emb0", [2, P, A_S], bf,
                                  kind="ExternalOutput")
        dbg_emb1 = nc.dram_tensor("dbg_emb1", [2, P, A_S], bf,
                                  kind="ExternalOutput")
        dbg_mT = nc.dram_tensor("dbg_mT", [P, 512], f32,
                                kind="ExternalOutput")
        dbg_x2 = nc.dram_tensor("dbg_x2", [2, P, A_S], bf,
                                kind="ExternalOutput")
        dbg_r = nc.dram_tensor("dbg_r", [P, 2 * B_S], f32,
                               kind="ExternalOutput")
        dbg_h = nc.dram_tensor("dbg_h", [N_ACTIONS, B_S], f32,
                               kind="ExternalOutput")

    with tile.TileContext(nc) as tc, ExitStack() as ctx:
        consts = ctx.enter_context(tc.tile_pool(name="consts", bufs=1))
        wpool = ctx.enter_context(tc.tile_pool(name="wpool", bufs=1))
        gpool = ctx.enter_context(tc.tile_pool(name="gath", bufs=2))
        epool = ctx.enter_context(tc.tile_pool(name="emb", bufs=2))
        e1pool = ctx.enter_context(tc.tile_pool(name="emb1", bufs=1))
        xpool = ctx.enter_context(tc.tile_pool(name="x2p", bufs=1))
        spool = ctx.enter_context(tc.tile_pool(name="small", bufs=2))
        main_ps = ctx.enter_context(
            tc.tile_pool(name="mps", bufs=4, space="PSUM"))
        tp_ps = ctx.enter_context(
            tc.tile_pool(name="tps", bufs=2, space="PSUM"))
        sm_ps = ctx.enter_context(
            tc.tile_pool(name="sps", bufs=2, space="PSUM"))

        ident = consts.tile([P, P], f32, tag="ident", name="ident")
        make_identity(nc, ident[:])

        wlT = wpool.tile([P, 2, 2, 2, P], bf, tag="wlT", name="wlT")
        nc.sync.dma_start(wlT[:], wlT_d[:])
        wrTn = wpool.tile([P, 2, 2, 2, P], bf, tag="wrTn", name="wrTn")
        nc.sync.dma_start(wrTn[:], wrTn_d[:])
        wrT = wpool.tile([P, 2, 2, 2, P], bf, tag="wrT", name="wrT")
        nc.sync.dma_start(wrT[:], wrT_d[:])
        bias_sb = wpool.tile([1, 2, 2, P], bf, tag="bias", name="bias")
        nc.sync.dma_start(bias_sb[:], bias_d[:])
        whT = wpool.tile([P, 2, N_ACTIONS], bf, tag="whT", name="whT")
        nc.sync.dma_start(whT[:], whT_d[:])
        bh_sb = wpool.tile([1, N_ACTIONS], bf, tag="bh", name="bh")
        nc.sync.dma_start(bh_sb[:], bh_d[:])
        ones_sb = wpool.tile([1, B_S], bf, tag="ones", name="ones")
        nc.sync.dma_start(ones_sb[:], ones_d[:])

        def tpack(src_aps, dst_ap):
            """PE-transpose [p<=128, w<=128] fp32 APs into one psum bank,
            then one ACT copy (w/ cast) into dst_ap (columns concatenated).
            Each src must have 128 partitions."""
            ps = tp_ps.tile([P, 512], f32, tag="tpack", name="tpack")
            col = 0
            for a in src_aps:
                w = a.shape[-1]
                nc.tensor.transpose(ps[:, col:col + w], a, ident[:])
                col += w
            nc.scalar.activation(dst_ap, ps[:, :col], AF.Copy)

        for s in range(NSUB):
            # ---------------- Phase A: stream pre-gathered slabs + bag-sum
            xsel = gpool.tile([P, A_S], bf, tag="xsel", name="xsel", bufs=1)
            nc.sync.dma_start(xsel[:], xsel_d[s])
            recipb = gpool.tile([P, A_S], bf, tag="recipb", name="recipb", bufs=1)
            nc.sync.dma_start(recipb[:], recip_d[s])
            s2_sb = gpool.tile([P, NCHUNK, W2], bf, tag="s2", name="s2")
            nc.sync.dma_start(s2_sb[:], s2_d[s])
            lenm1 = gpool.tile([1, B_S], bf, tag="lenm1", name="lenm1")
            nc.sync.dma_start(lenm1[:], lenm1_d[s])

            emb = [None] * NLAYERS
            emb[0] = [epool.tile([P, A_S], bf, tag=f"emb0_{t}", name=f"emb0_{t}")
                      for t in range(2)]
            emb[1] = [e1pool.tile([P, A_S], bf, tag=f"emb1_{t}", name=f"emb1_{t}")
                      for t in range(2)]
            emb[2] = [e1pool.tile([P, A_S], bf, tag=f"emb2_{t}", name=f"emb2_{t}")
                      for t in range(2)]

            # bag-sum of the 8 word planes happens inside the SDMA engines
            # (CCE ADD accumulate); no compute engine involved. CCE corrupts
            # descriptors >2048 elements, so accumulates go in 2048-wide
            # chunks; same-address RMW needs the sem-ordered separate ops.
            for t in range(2):
                nc.sync.dma_start(emb[0][t][:], slab_d[s, t, 0])
                for c0 in range(0, A_S, 2048):
                    c1 = min(c0 + 2048, A_S)
                    for k in range(1, BAG):
                        nc.gpsimd.dma_start(emb[0][t][:, c0:c1],
                                            slab_d[s, t, k, :, c0:c1],
                                            accum_op=OP.add)

            if _DEBUG and s == 0:
                for t in range(2):
                    nc.sync.dma_start(dbg_emb0[t], emb[0][t][:])

            # ---------------- helpers
            def segsum(src):
                """src = [t0, t1] bf16 [P, A_S] -> m^T psum [P, 512] f32:
                cols 0:256 = batches 0:128, cols 256:512 = batches 128:160
                (on partitions 0:32)."""
                grp = [spool.tile([P, NG], f32, tag=f"grp{t}", name=f"grp{t}",
                                  bufs=1)
                       for t in range(2)]
                for t in range(2):
                    nc.vector.tensor_reduce(
                        grp[t][:],
                        src[t][:].rearrange("p (g e) -> p g e", e=G),
                        axis=mybir.AxisListType.X, op=OP.add)
                gt = spool.tile([P, NCHUNK * EDIM], bf, tag="gt", name="gt")
                for c in range(NCHUNK):
                    tpack([grp[t][:, c * P:(c + 1) * P] for t in range(2)],
                          gt[:, c * EDIM:(c + 1) * EDIM])
                m_ps = sm_ps.tile([P, 512], f32, tag="sps", name="sps")
                for c in range(NCHUNK):
                    r0 = W2 * c if c < 4 else 0
                    dst = (m_ps[r0:r0 + W2, 0:EDIM] if c < 4
                           else m_ps[0:W2, EDIM:2 * EDIM])
                    nc.tensor.matmul(dst, lhsT=s2_sb[:, c, :],
                                     rhs=gt[:, c * EDIM:(c + 1) * EDIM],
                                     start=True, stop=True,
                                     skip_group_check=True,
                                     tile_position=(0, r0))
                return m_ps

            def m_to_sbuf(m_ps):
                mT = spool.tile([P, 512], f32, tag="mT", name="mT")
                nc.scalar.activation(mT[:, 0:EDIM], m_ps[:, 0:EDIM], AF.Copy)
                nc.scalar.activation(mT[0:W2, EDIM:2 * EDIM],
                                     m_ps[0:W2, EDIM:2 * EDIM], AF.Copy)
                return mT

            def m_dimmajor(mT_sb):
                """m^T sbuf -> mdm bf16 [P, 2(dt), B_S] (dim-major m)."""
                ps = sm_ps.tile([P, 512], f32, tag="sps", name="sps")
                for t in range(2):
                    nc.tensor.transpose(ps[:, t * B_S:t * B_S + P],
                                        mT_sb[:, t * P:(t + 1) * P],
                                        ident[:])
                    nc.tensor.transpose(
                        ps[:, t * B_S + P:t * B_S + B_S],
                        mT_sb[0:W2, EDIM + t * P:EDIM + (t + 1) * P],
                        ident[0:W2, 0:W2])
                out = spool.tile([P, 2 * B_S], bf, tag="mdm", name="mdm")
                nc.scalar.activation(out[:], ps[:, 0:2 * B_S], AF.Copy)
                return out

            # ---------------- layers 0, 1
            for i in range(2):
                x2 = [xpool.tile([P, A_S], bf, tag=f"x2_{t}", name=f"x2_{t}")
                      for t in range(2)]
                for t in range(2):
                    nc.vector.tensor_tensor(out=x2[t][:], in0=emb[i][t][:],
                                            in1=recipb[:], op=OP.mult)
                m_ps = segsum(emb[i])
                mT_sb_dbg = m_to_sbuf(m_ps)
                if _DEBUG and s == 0 and i == 0:
                    for t in range(2):
                        nc.sync.dma_start(dbg_x2[t], x2[t][:])
                    nc.sync.dma_start(dbg_mT[:], mT_sb_dbg[:])
                mdm = m_dimmajor(mT_sb_dbg)
                r_ps = sm_ps.tile([P, 512], f32, tag="sps", name="sps")
                for dt in range(2):
                    sl = r_ps[:, dt * B_S:(dt + 1) * B_S]
                    for kt in range(2):
                        nc.tensor.matmul(
                            sl, lhsT=wrT[:, i, kt, dt, :],
                            rhs=mdm[:, kt * B_S:(kt + 1) * B_S],
                            start=(kt == 0), stop=False)
                    nc.tensor.matmul(sl, lhsT=bias_sb[:, i, dt, :],
                                     rhs=lenm1[:], start=False, stop=True)
                r_sb = spool.tile([P, 2 * B_S], f32, tag="r_sb", name="r_sb")
                nc.scalar.activation(r_sb[:], r_ps[:, 0:2 * B_S], AF.Copy)
                if _DEBUG and s == 0 and i == 0:
                    nc.sync.dma_start(dbg_r[:], r_sb[:])
                # R^T at alignments 0 / 64 / 128 -> rt [P, 3, 256] bf16
                rt = spool.tile([P, 3, EDIM], bf, tag="rt", name="rt")
                nc.vector.memset(rt[:], 0.0)
                rt_ps = sm_ps.tile([P, 512], f32, tag="sps", name="sps")
                for dt in range(2):
                    nc.tensor.transpose(
                        rt_ps[:, dt * P:(dt + 1) * P],
                        r_sb[:, dt * B_S:dt * B_S + P], ident[:])
                nc.scalar.activation(rt[:, 0, :], rt_ps[:, 0:EDIM], AF.Copy)
                rt_ps2 = sm_ps.tile([P, 512], f32, tag="sps", name="sps")
                for dt in range(2):
                    nc.tensor.transpose(
                        rt_ps2[0:96, dt * P:(dt + 1) * P],
                        r_sb[:, dt * B_S + 64:dt * B_S + B_S], ident[:])
                    nc.tensor.transpose(
                        rt_ps2[0:W2, EDIM + dt * P:EDIM + dt * P + P],
                        r_sb[:, dt * B_S + P:dt * B_S + B_S], ident[:])
                nc.scalar.activation(rt[0:96, 1, :], rt_ps2[0:96, 0:EDIM],
                                     AF.Copy)
                nc.scalar.activation(rt[0:W2, 2, :],
                                     rt_ps2[0:W2, EDIM:2 * EDIM], AF.Copy)
                # main matmuls
                for j in range(NBG):
                    al = min(j // 4, 2)
                    js = slice(j * BG_AGENTS, (j + 1) * BG_AGENTS)
                    for dt in range(2):
                        ps = main_ps.tile([P, BG_AGENTS], f32, tag="main", name="main")
                        for kt in range(2):
                            nc.tensor.matmul(
                                ps[:], lhsT=wlT[:, i, kt, dt, :],
                                rhs=emb[i][kt][:, js],
                                start=(kt == 0), stop=False)
                        for kt in range(2):
                            nc.tensor.matmul(
                                ps[:], lhsT=wrTn[:, i, kt, dt, :],
                                rhs=x2[kt][:, js], start=False, stop=False)
                        nc.tensor.matmul(
                            ps[:], lhsT=rt[:, al, dt * P:(dt + 1) * P],
                            rhs=xsel[:, js], start=False, stop=True)
                        nc.scalar.activation(emb[i + 1][dt][:, js], ps[:],
                                             AF.Relu)
                if _DEBUG and s == 0 and i == 0:
                    for t in range(2):
                        nc.sync.dma_start(dbg_emb1[t], emb[1][t][:])

            # ---------------- final segsum + head + log_softmax
            m_ps = segsum(emb[2])
            mdm = m_dimmajor(m_to_sbuf(m_ps))
            h_ps = sm_ps.tile([P, 512], f32, tag="sps", name="sps")
            hsl = h_ps[0:N_ACTIONS, 0:B_S]
            for kt in range(2):
                nc.tensor.matmul(hsl, lhsT=whT[:, kt, :],
                                 rhs=mdm[:, kt * B_S:(kt + 1) * B_S],
                                 start=(kt == 0), stop=False)
            nc.tensor.matmul(hsl, lhsT=bh_sb[:], rhs=ones_sb[:],
                             start=False, stop=True)
            h_sb = spool.tile([N_ACTIONS, B_S], f32, tag="h_sb", name="h_sb")
            nc.scalar.activation(h_sb[:], hsl, AF.Copy)
            if _DEBUG and s == 0:
                nc.sync.dma_start(dbg_h[:], h_sb[:])
            lg_ps = sm_ps.tile([P, 512], f32, tag="sps", name="sps")
            nc.tensor.transpose(lg_ps[:, 0:N_ACTIONS], h_sb[:, 0:P],
                                ident[0:N_ACTIONS, 0:N_ACTIONS])
            nc.tensor.transpose(lg_ps[0:W2, N_ACTIONS:2 * N_ACTIONS],
                                h_sb[:, P:B_S],
                                ident[0:N_ACTIONS, 0:N_ACTIONS])
            lg = spool.tile([P, 2 * N_ACTIONS], f32, tag="lg_sb", name="lg_sb")
            nc.scalar.activation(lg[:, 0:N_ACTIONS], lg_ps[:, 0:N_ACTIONS],
                                 AF.Copy)
            nc.scalar.activation(lg[0:W2, N_ACTIONS:2 * N_ACTIONS],
                                 lg_ps[0:W2, N_ACTIONS:2 * N_ACTIONS],
                                 AF.Copy)
            for part in range(2):
                rows = P if part == 0 else B_S - P
                src = lg[0:rows, part * N_ACTIONS:(part + 1) * N_ACTIONS]
                mx = spool.tile([P, 1], f32, tag="mx", name="mx")
                nc.vector.tensor_reduce(mx[0:rows, :], src,
                                        axis=mybir.AxisListType.X,
                                        op=OP.max)
                shv = spool.tile([P, N_ACTIONS], f32, tag="shift", name="shift")
                nc.vector.tensor_tensor(
                    out=shv[0:rows, :], in0=src,
                    in1=mx[0:rows, :].to_broadcast([rows, N_ACTIONS]),
                    op=OP.subtract)
                ex = spool.tile([P, N_ACTIONS], f32, tag="ex", name="ex")
                se = spool.tile([P, 1], f32, tag="se", name="se")
                nc.scalar.activation(ex[0:rows, :], shv[0:rows, :], AF.Exp,
                                     accum_out=se[0:rows, :])
                lse = spool.tile([P, 1], f32, tag="lse", name="lse")
                nc.scalar.activation(lse[0:rows, :], se[0:rows, :], AF.Ln)
                res = spool.tile([P, N_ACTIONS], f32, tag="res", name="res")
                nc.vector.tensor_tensor(
                    out=res[0:rows, :], in0=shv[0:rows, :],
                    in1=lse[0:rows, :].to_broadcast([rows, N_ACTIONS]),
                    op=OP.subtract)
                nc.sync.dma_start(out_d[s, part * P:part * P + rows, :],
                                  res[0:rows, :])
    nc.compile()
    return nc


# ================================================================== kernel
def kernel(**inputs):
    per_core, out_map, _ = _build_host_inputs(
        inputs["x"], inputs["batch_idx"], inputs["batch_len"],
        inputs["emb_table"], inputs["W0"], inputs["b0"], inputs["W1"],
        inputs["b1"], inputs["Wh"], inputs["bh"])

    if "prog" not in _PROGRAM_CACHE:
        _PROGRAM_CACHE["prog"] = _build_program()
    nc = _PROGRAM_CACHE["prog"]

    from concourse.bass_utils import run_bass_kernel_spmd
    res = run_bass_kernel_spmd(nc, per_core, core_ids=list(range(N_CORES)))

    logp = np.zeros((N_BATCHES, N_ACTIONS), np.float32)
    for core in range(N_CORES):
        out = np.asarray(res.results[core]["out"], np.float32)
        for s in range(NSUB):
            sh = core * NSUB + s
            v = out_map[sh] >= 0
            logp[out_map[sh][v]] = out[s][v]
    return (logp,)



# revision 16
# speedup vs baseline: 1.3248x; 1.0755x over previous
"""Trainium2 Bass kernel for nn_Commnet (gnn_message_passing).

kernel(**inputs) takes FULL unsharded numpy inputs, returns (logp [4096,16],)
computed across 8 NeuronCores (SPMD single program; per-core structure is
carried entirely in input tensors).

Sharding: batches bin-packed into 32 sub-shards (4/core); each sub-shard =
10 batch-groups of 16 batch slots padded to exactly 512 agent slots, so every
512-agent matmul block has a static 16-batch selector window -> the program
is identical on all cores.

EmbeddingBag(mean): table cast to bf16 pre-scaled by 1/8 on host (exact);
the host pre-permutes the per-agent word rows into dim-major slabs
slabT[t][dim, agent*8+word] (a pure row gather/layout, no arithmetic) so the
device streams them with large contiguous HWDGE DMAs at line rate; the
bag-sum is a DVE group-reduce(8) along the free dim, directly producing the
dim-major agent embeddings (no PE transposes, no SWDGE descriptor storm).

Middle layers: emb' = relu(W_l@emb - W_r@(emb*recip) + R'@sel) where
R' = W_r@m + b (x) (len-0.99999) folds expansion+bias; sel is a banded
one-hot*recip selector (1 extra k-tile per block). m = segment sum via DVE
group-sum(8) -> PE transpose -> banded matmuls into disjoint PSUM windows.
Padded agents stay exactly 0 through all layers.
"""

from contextlib import ExitStack

import numpy as np
import ml_dtypes

import concourse.bass as bass
import concourse.bacc as bacc
import concourse.tile as tile
from concourse import mybir
from concourse.masks import make_identity

N_WORDS = 32000
EDIM = 256
N_AGENTS = 131072
BAG = 8
N_BATCHES = 4096
N_ACTIONS = 16
NLAYERS = 3

N_CORES = 8
NSUB = 4                  # sub-shards per core
NSHARD = N_CORES * NSUB   # 32
NBG = 10                  # batch-groups per sub-shard
BG_BATCHES = 16
BG_AGENTS = 512
B_S = NBG * BG_BATCHES    # 160
A_S = NBG * BG_AGENTS     # 5120
G = 8
NG = A_S // G             # 640
NCHUNK = NG // 128        # 5
W2 = 2 * BG_BATCHES       # 32
P = 128
ZPAD_ROW = N_WORDS
DT = mybir.dt
AF = mybir.ActivationFunctionType
OP = mybir.AluOpType

_PROGRAM_CACHE = {}
_DEBUG = False


# ================================================================ host prep
def _pack_batches(counts):
    padded = ((counts + G - 1) // G) * G
    ngroups = NSHARD * NBG
    cap = np.full(ngroups, BG_AGENTS, dtype=np.int64)
    slots = np.full(ngroups, BG_BATCHES, dtype=np.int64)
    members = [[] for _ in range(ngroups)]
    for b in np.argsort(-padded, kind="stable"):
        ok = (cap >= padded[b]) & (slots > 0)
        if not ok.any():
            raise RuntimeError("bin packing failed")
        g = int(np.argmax(np.where(ok, cap, -1)))
        members[g].append(int(b))
        cap[g] -= padded[b]
        slots[g] -= 1
    return members, padded


def _build_host_inputs(x, batch_idx, batch_len, emb_table, W0, b0, W1, b1,
                       Wh, bh):
    bf16 = ml_dtypes.bfloat16
    x = np.asarray(x, dtype=np.int64)
    batch_idx = np.asarray(batch_idx, dtype=np.int64)
    batch_len64 = np.asarray(batch_len, dtype=np.float64)

    counts = np.bincount(batch_idx, minlength=N_BATCHES).astype(np.int64)
    starts = np.concatenate([[0], np.cumsum(counts)[:-1]])
    members, padded = _pack_batches(counts)

    # table rows pre-scaled by 1/8 (exact), bf16, with a zero pad row;
    # uint16 view for fast host-side take/transpose.
    tblu = np.concatenate(
        [np.asarray(emb_table, np.float32) / 8.0,
         np.zeros((1, EDIM), np.float32)], 0).astype(bf16).view(np.uint16)
    # dim-major word-plane slabs: slab_all[sh, t, k, p, a] =
    # table[x[a*BAG+k], t*128+p] / 8 (zeros for padded agent slots).
    # The device bag-sums the 8 k-planes via SWDGE accumulate-DMA (CCE ADD).
    slab_all = np.empty((NSHARD, 2, BAG, P, A_S), dtype=np.uint16)
    xsel_all = np.zeros((NSHARD, P, A_S), dtype=bf16)
    recip_all = np.zeros((NSHARD, P, A_S), dtype=bf16)
    s2_all = np.zeros((NSHARD, P, NCHUNK, W2), dtype=bf16)
    lenm1_all = np.zeros((NSHARD, 1, B_S), dtype=bf16)
    gcnt_all = np.full((NSHARD, 1, NBG // 2), BG_AGENTS * 2, dtype=np.int32)
    out_map = np.full((NSHARD, B_S), -1, dtype=np.int64)
    dbg_slots = []
    recip_f = (1.0 / (batch_len64 - 0.99999)).astype(np.float32)

    for sh in range(NSHARD):
        idx_flat = np.full((BAG, A_S), ZPAD_ROW, dtype=np.int64)
        a_of_slot = np.full(A_S, -1, dtype=np.int64)
        b_of_slot = np.full(A_S, -1, dtype=np.int64)
        grp_content = np.zeros(NBG, np.int64)
        for bg in range(NBG):
            pos = bg * BG_AGENTS
            for sl, b in enumerate(members[sh * NBG + bg]):
                lb = bg * BG_BATCHES + sl
                out_map[sh, lb] = b
                lenm1_all[sh, 0, lb] = np.float32(batch_len64[b] - 0.99999)
                n = int(counts[b])
                a_of_slot[pos:pos + n] = np.arange(starts[b], starts[b] + n)
                b_of_slot[pos:pos + n] = lb
                pos += int(padded[b])
            grp_content[bg] = pos - bg * BG_AGENTS
        slots = np.nonzero(a_of_slot >= 0)[0]
        ags = a_of_slot[slots]
        for k in range(BAG):
            idx_flat[k, slots] = x[ags * BAG + k]

        # pre-gather + transpose to dim-major word planes
        for k in range(BAG):
            gath = tblu[idx_flat[k]]               # [A_S, 256]
            slab_all[sh, :, k] = gath.T.reshape(2, P, A_S)

        lb_real = b_of_slot[slots]
        rec = recip_f[out_map[sh, lb_real]]
        recip_row = np.zeros(A_S, np.float32)
        recip_row[slots] = rec
        recip_all[sh] = np.broadcast_to(recip_row.astype(bf16), (P, A_S))

        j_of_slot = slots // BG_AGENTS
        w0_al = np.where(j_of_slot >= 8, 128, (j_of_slot // 4) * 64)
        r = lb_real - w0_al
        assert (r >= 0).all() and (r < P).all()
        xs = np.zeros((P, A_S), np.float32)
        xs[r, slots] = rec
        xsel_all[sh] = xs.astype(bf16)

        dbg_slots.append((a_of_slot.copy(), b_of_slot.copy()))
        g_b = b_of_slot[::G]
        for c in range(NCHUNK):
            gl = np.arange(P)
            gb = g_b[c * P + gl]
            v = gb >= 0
            w = gb[v] - W2 * c
            assert (w >= 0).all() and (w < W2).all()
            s2_all[sh][gl[v], c, w] = 1.0

    W0 = np.asarray(W0, np.float32)
    W1 = np.asarray(W1, np.float32)
    wl = np.stack([W0[:, :EDIM].T, W1[:, :EDIM].T])   # [layer, 256k, 256d]
    wr = np.stack([W0[:, EDIM:].T, W1[:, EDIM:].T])

    def tiles(w):  # [2,256,256] -> [128, 2(layer), 2(kt), 2(dt), 128]
        t = w.reshape(2, 2, P, 2, P).transpose(2, 0, 1, 3, 4)
        return np.ascontiguousarray(t).astype(bf16)

    host = {
        "wlT": tiles(wl),
        "wrTn": tiles(-wr),
        "wrT": tiles(wr),
        "bias": np.ascontiguousarray(
            np.stack([np.asarray(b0, np.float32), np.asarray(b1, np.float32)])
            .reshape(2, 2, P).transpose(1, 0, 2)[None]  # wrong axis order?
        ),
        "whT": np.ascontiguousarray(
            np.asarray(Wh, np.float32).T.reshape(2, P, N_ACTIONS)
            .transpose(1, 0, 2)).astype(bf16),
        "bh": np.asarray(bh, np.float32).reshape(1, N_ACTIONS).astype(bf16),
        "ones_b": np.ones((1, B_S), bf16),
    }
    # bias layout: [1, 2(layer), 2(dt), 128]
    bias = np.stack([np.asarray(b0, np.float32),
                     np.asarray(b1, np.float32)]).reshape(2, 2, P)
    host["bias"] = bias[None].astype(bf16)

    slab_bf = slab_all.view(bf16)
    per_core = []
    for core in range(N_CORES):
        s0 = core * NSUB
        m = dict(host)
        m["slabT"] = slab_bf[s0:s0 + NSUB]
        m["xsel"] = xsel_all[s0:s0 + NSUB]
        m["recipb"] = recip_all[s0:s0 + NSUB]
        m["s2"] = s2_all[s0:s0 + NSUB]
        m["lenm1"] = lenm1_all[s0:s0 + NSUB]
        per_core.append(m)
    return per_core, out_map, dbg_slots


# ============================================================ device program
def _build_program():
    nc = bacc.Bacc("TRN2")
    bf, f32, i32 = DT.bfloat16, DT.float32, DT.int32

    slab_d = nc.dram_tensor("slabT", [NSUB, 2, BAG, P, A_S], bf,
                            kind="ExternalInput")
    xsel_d = nc.dram_tensor("xsel", [NSUB, P, A_S], bf, kind="ExternalInput")
    recip_d = nc.dram_tensor("recipb", [NSUB, P, A_S], bf,
                             kind="ExternalInput")
    s2_d = nc.dram_tensor("s2", [NSUB, P, NCHUNK, W2], bf,
                          kind="ExternalInput")
    lenm1_d = nc.dram_tensor("lenm1", [NSUB, 1, B_S], bf,
                             kind="ExternalInput")
    wlT_d = nc.dram_tensor("wlT", [P, 2, 2, 2, P], bf, kind="ExternalInput")
    wrTn_d = nc.dram_tensor("wrTn", [P, 2, 2, 2, P], bf, kind="ExternalInput")
    wrT_d = nc.dram_tensor("wrT", [P, 2, 2, 2, P], bf, kind="ExternalInput")
    bias_d = nc.dram_tensor("bias", [1, 2, 2, P], bf, kind="ExternalInput")
    whT_d = nc.dram_tensor("whT", [P, 2, N_ACTIONS], bf,
                           kind="ExternalInput")
    bh_d = nc.dram_tensor("bh", [1, N_ACTIONS], bf, kind="ExternalInput")
    ones_d = nc.dram_tensor("ones_b", [1, B_S], bf, kind="ExternalInput")
    out_d = nc.dram_tensor("out", [NSUB, B_S, N_ACTIONS], f32,
                           kind="ExternalOutput")
    if _DEBUG:
        dbg_emb0 = nc.dram_tensor("dbg_emb0", [2, P, A_S], bf,
                                  kind="ExternalOutput")
        dbg_emb1 = nc.dram_tensor("dbg_emb1", [2, P, A_S], bf,
                                  kind="ExternalOutput")
        dbg_mT = nc.dram_tensor("dbg_mT", [P, 512], f32,
                                kind="ExternalOutput")
        dbg_x2 = nc.dram_tensor("dbg_x2", [2, P, A_S], bf,
                                kind="ExternalOutput")
        dbg_r = nc.dram_tensor("dbg_r", [P, 2 * B_S], f32,
                               kind="ExternalOutput")
        dbg_h = nc.dram_tensor("dbg_h", [N_ACTIONS, B_S], f32,
                               kind="ExternalOutput")

    with tile.TileContext(nc) as tc, ExitStack() as ctx:
        consts = ctx.enter_context(tc.tile_pool(name="consts", bufs=1))
        wpool = ctx.enter_context(tc.tile_pool(name="wpool", bufs=1))
        gpool = ctx.enter_context(tc.tile_pool(name="gath", bufs=2))
        epool = ctx.enter_context(tc.tile_pool(name="emb", bufs=2))
        e1pool = ctx.enter_context(tc.tile_pool(name="emb1", bufs=1))
        xpool = ctx.enter_context(tc.tile_pool(name="x2p", bufs=1))
        spool = ctx.enter_context(tc.tile_pool(name="small", bufs=2))
        main_ps = ctx.enter_context(
            tc.tile_pool(name="mps", bufs=4, space="PSUM"))
        tp_ps = ctx.enter_context(
            tc.tile_pool(name="tps", bufs=2, space="PSUM"))
        sm_ps = ctx.enter_context(
            tc.tile_pool(name="sps", bufs=2, space="PSUM"))

        ident = consts.tile([P, P], f32, tag="ident", name="ident")
        make_identity(nc, ident[:])

        wlT = wpool.tile([P, 2, 2, 2, P], bf, tag="wlT", name="wlT")
        nc.sync.dma_start(wlT[:], wlT_d[:])
        wrTn = wpool.tile([P, 2, 2, 2, P], bf, tag="wrTn", name="wrTn")
        nc.sync.dma_start(wrTn[:], wrTn_d[:])
        wrT = wpool.tile([P, 2, 2, 2, P], bf, tag="wrT", name="wrT")
        nc.sync.dma_start(wrT[:], wrT_d[:])
        bias_sb = wpool.tile([1, 2, 2, P], bf, tag="bias", name="bias")
        nc.sync.dma_start(bias_sb[:], bias_d[:])
        whT = wpool.tile([P, 2, N_ACTIONS], bf, tag="whT", name="whT")
        nc.sync.dma_start(whT[:], whT_d[:])
        bh_sb = wpool.tile([1, N_ACTIONS], bf, tag="bh", name="bh")
        nc.sync.dma_start(bh_sb[:], bh_d[:])
        ones_sb = wpool.tile([1, B_S], bf, tag="ones", name="ones")
        nc.sync.dma_start(ones_sb[:], ones_d[:])

        def tpack(src_aps, dst_ap):
            """PE-transpose [p<=128, w<=128] fp32 APs into one psum bank,
            then one ACT copy (w/ cast) into dst_ap (columns concatenated).
            Each src must have 128 partitions."""
            ps = tp_ps.tile([P, 512], f32, tag="tpack", name="tpack")
            col = 0
            for a in src_aps:
                w = a.shape[-1]
                nc.tensor.transpose(ps[:, col:col + w], a, ident[:])
                col += w
            nc.scalar.activation(dst_ap, ps[:, :col], AF.Copy)

        for s in range(NSUB):
            # ---------------- Phase A: stream pre-gathered slabs + bag-sum
            xsel = gpool.tile([P, A_S], bf, tag="xsel", name="xsel", bufs=1)
            nc.sync.dma_start(xsel[:], xsel_d[s])
            recipb = gpool.tile([P, A_S], bf, tag="recipb", name="recipb", bufs=1)
            nc.sync.dma_start(recipb[:], recip_d[s])
            s2_sb = gpool.tile([P, NCHUNK, W2], bf, tag="s2", name="s2")
            nc.sync.dma_start(s2_sb[:], s2_d[s])
            lenm1 = gpool.tile([1, B_S], bf, tag="lenm1", name="lenm1")
            nc.sync.dma_start(lenm1[:], lenm1_d[s])

            emb = [None] * NLAYERS
            emb[0] = [epool.tile([P, A_S], bf, tag=f"emb0_{t}", name=f"emb0_{t}")
                      for t in range(2)]
            emb[1] = [e1pool.tile([P, A_S], bf, tag=f"emb1_{t}", name=f"emb1_{t}")
                      for t in range(2)]
            emb[2] = [e1pool.tile([P, A_S], bf, tag=f"emb2_{t}", name=f"emb2_{t}")
                      for t in range(2)]

            # bag-sum of the 8 word planes, split across units: plane 0
            # lands via plain HWDGE, plane 1 accumulates inside the SDMA
            # engines (CCE ADD; descriptors >2048 elements corrupt, so
            # 2048-wide chunks; same-address RMW needs sem-ordered ops),
            # planes 2-7 go through a DVE pairwise add tree (2x bf16 uop;
            # tensor_reduce only has the 1x uop).
            CW = A_S // 4
            for t in range(2):
                nc.sync.dma_start(emb[0][t][:], slab_d[s, t, 0])
                for c0 in range(0, A_S, 2048):
                    c1 = min(c0 + 2048, A_S)
                    nc.gpsimd.dma_start(emb[0][t][:, c0:c1],
                                        slab_d[s, t, 1, :, c0:c1],
                                        accum_op=OP.add)
                for c in range(4):
                    st = gpool.tile([P, 6, CW], bf, tag="stage",
                                    name="stage", bufs=2)
                    cs = slice(c * CW, (c + 1) * CW)
                    for k in range(6):
                        nc.sync.dma_start(st[:, k, :],
                                          slab_d[s, t, 2 + k, :, cs])
                    for (d, a, b) in ((0, 0, 1), (2, 2, 3), (4, 4, 5),
                                     (0, 0, 2), (0, 0, 4)):
                        nc.vector.tensor_tensor(out=st[:, d, :],
                                                in0=st[:, a, :],
                                                in1=st[:, b, :], op=OP.add)
                    nc.vector.tensor_tensor(out=emb[0][t][:, cs],
                                            in0=emb[0][t][:, cs],
                                            in1=st[:, 0, :], op=OP.add)

            if _DEBUG and s == 0:
                for t in range(2):
                    nc.sync.dma_start(dbg_emb0[t], emb[0][t][:])

            # ---------------- helpers
            def segsum(src):
                """src = [t0, t1] bf16 [P, A_S] -> m^T psum [P, 512] f32:
                cols 0:256 = batches 0:128, cols 256:512 = batches 128:160
                (on partitions 0:32)."""
                grp = [spool.tile([P, NG], f32, tag=f"grp{t}", name=f"grp{t}",
                                  bufs=1)
                       for t in range(2)]
                for t in range(2):
                    nc.vector.tensor_reduce(
                        grp[t][:],
                        src[t][:].rearrange("p (g e) -> p g e", e=G),
                        axis=mybir.AxisListType.X, op=OP.add)
                gt = spool.tile([P, NCHUNK * EDIM], bf, tag="gt", name="gt")
                for c in range(NCHUNK):
                    tpack([grp[t][:, c * P:(c + 1) * P] for t in range(2)],
                          gt[:, c * EDIM:(c + 1) * EDIM])
                m_ps = sm_ps.tile([P, 512], f32, tag="sps", name="sps")
                for c in range(NCHUNK):
                    r0 = W2 * c if c < 4 else 0
                    dst = (m_ps[r0:r0 + W2, 0:EDIM] if c < 4
                           else m_ps[0:W2, EDIM:2 * EDIM])
                    nc.tensor.matmul(dst, lhsT=s2_sb[:, c, :],
                                     rhs=gt[:, c * EDIM:(c + 1) * EDIM],
                                     start=True, stop=True,
                                     skip_group_check=True,
                                     tile_position=(0, r0))
                return m_ps

            def m_to_sbuf(m_ps):
                mT = spool.tile([P, 512], f32, tag="mT", name="mT")
                nc.scalar.activation(mT[:, 0:EDIM], m_ps[:, 0:EDIM], AF.Copy)
                nc.scalar.activation(mT[0:W2, EDIM:2 * EDIM],
                                     m_ps[0:W2, EDIM:2 * EDIM], AF.Copy)
                return mT

            def m_dimmajor(mT_sb):
                """m^T sbuf -> mdm bf16 [P, 2(dt), B_S] (dim-major m)."""
                ps = sm_ps.tile([P, 512], f32, tag="sps", name="sps")
                for t in range(2):
                    nc.tensor.transpose(ps[:, t * B_S:t * B_S + P],
                                        mT_sb[:, t * P:(t + 1) * P],
                                        ident[:])
                    nc.tensor.transpose(
                        ps[:, t * B_S + P:t * B_S + B_S],
                        mT_sb[0:W2, EDIM + t * P:EDIM + (t + 1) * P],
                        ident[0:W2, 0:W2])
                out = spool.tile([P, 2 * B_S], bf, tag="mdm", name="mdm")
                nc.scalar.activation(out[:], ps[:, 0:2 * B_S], AF.Copy)
                return out

            # ---------------- layers 0, 1
            for i in range(2):
                x2 = [xpool.tile([P, A_S], bf, tag=f"x2_{t}", name=f"x2_{t}")
                      for t in range(2)]
                for t in range(2):
                    nc.vector.tensor_tensor(out=x2[t][:], in0=emb[i][t][:],
                                            in1=recipb[:], op=OP.mult)
                m_ps = segsum(emb[i])
                mT_sb_dbg = m_to_sbuf(m_ps)
                if _DEBUG and s == 0 and i == 0:
                    for t in range(2):
                        nc.sync.dma_start(dbg_x2[t], x2[t][:])
                    nc.sync.dma_start(dbg_mT[:], mT_sb_dbg[:])
                mdm = m_dimmajor(mT_sb_dbg)
                r_ps = sm_ps.tile([P, 512], f32, tag="sps", name="sps")
                for dt in range(2):
                    sl = r_ps[:, dt * B_S:(dt + 1) * B_S]
                    for kt in range(2):
                        nc.tensor.matmul(
                            sl, lhsT=wrT[:, i, kt, dt, :],
                            rhs=mdm[:, kt * B_S:(kt + 1) * B_S],
                            start=(kt == 0), stop=False)
                    nc.tensor.matmul(sl, lhsT=bias_sb[:, i, dt, :],
                                     rhs=lenm1[:], start=False, stop=True)
                r_sb = spool.tile([P, 2 * B_S], f32, tag="r_sb", name="r_sb")
                nc.scalar.activation(r_sb[:], r_ps[:, 0:2 * B_S], AF.Copy)
                if _DEBUG and s == 0 and i == 0:
                    nc.sync.dma_start(dbg_r[:], r_sb[:])
                # R^T at alignments 0 / 64 / 128 -> rt [P, 3, 256] bf16
                rt = spool.tile([P, 3, EDIM], bf, tag="rt", name="rt")
                nc.vector.memset(rt[:], 0.0)
                rt_ps = sm_ps.tile([P, 512], f32, tag="sps", name="sps")
                for dt in range(2):
                    nc.tensor.transpose(
                        rt_ps[:, dt * P:(dt + 1) * P],
                        r_sb[:, dt * B_S:dt * B_S + P], ident[:])
                nc.scalar.activation(rt[:, 0, :], rt_ps[:, 0:EDIM], AF.Copy)
                rt_ps2 = sm_ps.tile([P, 512], f32, tag="sps", name="sps")
                for dt in range(2):
                    nc.tensor.transpose(
                        rt_ps2[0:96, dt * P:(dt + 1) * P],
                        r_sb[:, dt * B_S + 64:dt * B_S + B_S], ident[:])
                    nc.tensor.transpose(
                        rt_ps2[0:W2, EDIM + dt * P:EDIM + dt * P + P],
                        r_sb[:, dt * B_S + P:dt * B_S + B_S], ident[:])
                nc.scalar.activation(rt[0:96, 1, :], rt_ps2[0:96, 0:EDIM],
                                     AF.Copy)
                nc.scalar.activation(rt[0:W2, 2, :],
                                     rt_ps2[0:W2, EDIM:2 * EDIM], AF.Copy)
                # main matmuls
                for j in range(NBG):
                    al = min(j // 4, 2)
                    js = slice(j * BG_AGENTS, (j + 1) * BG_AGENTS)
                    for dt in range(2):
                        ps = main_ps.tile([P, BG_AGENTS], f32, tag="main", name="main")
                        for kt in range(2):
                            nc.tensor.matmul(
                                ps[:], lhsT=wlT[:, i, kt, dt, :],
                                rhs=emb[i][kt][:, js],
                                start=(kt == 0), stop=False)
                        for kt in range(2):
                            nc.tensor.matmul(
                                ps[:], lhsT=wrTn[:, i, kt, dt, :],
                                rhs=x2[kt][:, js], start=False, stop=False)
                        nc.tensor.matmul(
                            ps[:], lhsT=rt[:, al, dt * P:(dt + 1) * P],
                            rhs=xsel[:, js], start=False, stop=True)
                        nc.scalar.activation(emb[i + 1][dt][:, js], ps[:],
                                             AF.Relu)
                if _DEBUG and s == 0 and i == 0:
                    for t in range(2):
                        nc.sync.dma_start(dbg_emb1[t], emb[1][t][:])

            # ---------------- final segsum + head + log_softmax
            m_ps = segsum(emb[2])
            mdm = m_dimmajor(m_to_sbuf(m_ps))
            h_ps = sm_ps.tile([P, 512], f32, tag="sps", name="sps")
            hsl = h_ps[0:N_ACTIONS, 0:B_S]
            for kt in range(2):
                nc.tensor.matmul(hsl, lhsT=whT[:, kt, :],
                                 rhs=mdm[:, kt * B_S:(kt + 1) * B_S],
                                 start=(kt == 0), stop=False)
            nc.tensor.matmul(hsl, lhsT=bh_sb[:], rhs=ones_sb[:],
                             start=False, stop=True)
            h_sb = spool.tile([N_ACTIONS, B_S], f32, tag="h_sb", name="h_sb")
            nc.scalar.activation(h_sb[:], hsl, AF.Copy)
            if _DEBUG and s == 0:
                nc.sync.dma_start(dbg_h[:], h_sb[:])
            lg_ps = sm_ps.tile([P, 512], f32, tag="sps", name="sps")
            nc.tensor.transpose(lg_ps[:, 0:N_ACTIONS], h_sb[:, 0:P],
                                ident[0:N_ACTIONS, 0:N_ACTIONS])
            nc.tensor.transpose(lg_ps[0:W2, N_ACTIONS:2 * N_ACTIONS],
                                h_sb[:, P:B_S],
                                ident[0:N_ACTIONS, 0:N_ACTIONS])
            lg = spool.tile([P, 2 * N_ACTIONS], f32, tag="lg_sb", name="lg_sb")
            nc.scalar.activation(lg[:, 0:N_ACTIONS], lg_ps[:, 0:N_ACTIONS],
                                 AF.Copy)
            nc.scalar.activation(lg[0:W2, N_ACTIONS:2 * N_ACTIONS],
                                 lg_ps[0:W2, N_ACTIONS:2 * N_ACTIONS],
                                 AF.Copy)
            for part in range(2):
                rows = P if part == 0 else B_S - P
                src = lg[0:rows, part * N_ACTIONS:(part + 1) * N_ACTIONS]
                mx = spool.tile([P, 1], f32, tag="mx", name="mx")
                nc.vector.tensor_reduce(mx[0:rows, :], src,
                                        axis=mybir.AxisListType.X,
                                        op=OP.max)
                shv = spool.tile([P, N_ACTIONS], f32, tag="shift", name="shift")
                nc.vector.tensor_tensor(
                    out=shv[0:rows, :], in0=src,
                    in1=mx[0:rows, :].to_broadcast([rows, N_ACTIONS]),
                    op=OP.subtract)
                ex = spool.tile([P, N_ACTIONS], f32, tag="ex", name="ex")
                se = spool.tile([P, 1], f32, tag="se", name="se")
                nc.scalar.activation(ex[0:rows, :], shv[0:rows, :], AF.Exp,
                                     accum_out=se[0:rows, :])
                lse = spool.tile([P, 1], f32, tag="lse", name="lse")
                nc.scalar.activation(lse[0:rows, :], se[0:rows, :], AF.Ln)
                res = spool.tile([P, N_ACTIONS], f32, tag="res", name="res")
                nc.vector.tensor_tensor(
                    out=res[0:rows, :], in0=shv[0:rows, :],
                    in1=lse[0:rows, :].to_broadcast([rows, N_ACTIONS]),
                    op=OP.subtract)
                nc.sync.dma_start(out_d[s, part * P:part * P + rows, :],
                                  res[0:rows, :])
    nc.compile()
    return nc


# ================================================================== kernel
def kernel(**inputs):
    per_core, out_map, _ = _build_host_inputs(
        inputs["x"], inputs["batch_idx"], inputs["batch_len"],
        inputs["emb_table"], inputs["W0"], inputs["b0"], inputs["W1"],
        inputs["b1"], inputs["Wh"], inputs["bh"])

    if "prog" not in _PROGRAM_CACHE:
        _PROGRAM_CACHE["prog"] = _build_program()
    nc = _PROGRAM_CACHE["prog"]

    from concourse.bass_utils import run_bass_kernel_spmd
    res = run_bass_kernel_spmd(nc, per_core, core_ids=list(range(N_CORES)))

    logp = np.zeros((N_BATCHES, N_ACTIONS), np.float32)
    for core in range(N_CORES):
        out = np.asarray(res.results[core]["out"], np.float32)
        for s in range(NSUB):
            sh = core * NSUB + s
            v = out_map[sh] >= 0
            logp[out_map[sh][v]] = out[s][v]
    return (logp,)



# revision 19
# speedup vs baseline: 1.3922x; 1.0508x over previous
"""Trainium2 Bass kernel for nn_Commnet (gnn_message_passing).

kernel(**inputs) takes FULL unsharded numpy inputs, returns (logp [4096,16],)
computed across 8 NeuronCores (SPMD single program; per-core structure is
carried entirely in input tensors).

Sharding: batches bin-packed into 32 sub-shards (4/core); each sub-shard =
10 batch-groups of 16 batch slots padded to exactly 512 agent slots, so every
512-agent matmul block has a static 16-batch selector window -> the program
is identical on all cores.

EmbeddingBag(mean): table cast to bf16 pre-scaled by 1/8 on host (exact);
the host pre-permutes the per-agent word rows into dim-major slabs
slabT[t][dim, agent*8+word] (a pure row gather/layout, no arithmetic) so the
device streams them with large contiguous HWDGE DMAs at line rate; the
bag-sum is a DVE group-reduce(8) along the free dim, directly producing the
dim-major agent embeddings (no PE transposes, no SWDGE descriptor storm).

Middle layers: emb' = relu(W_l@emb - W_r@(emb*recip) + R'@sel) where
R' = W_r@m + b (x) (len-0.99999) folds expansion+bias; sel is a banded
one-hot*recip selector (1 extra k-tile per block). m = segment sum via DVE
group-sum(8) -> PE transpose -> banded matmuls into disjoint PSUM windows.
Padded agents stay exactly 0 through all layers.
"""

from contextlib import ExitStack

import numpy as np
import ml_dtypes

import concourse.bass as bass
import concourse.bacc as bacc
import concourse.tile as tile
from concourse import mybir
from concourse.masks import make_identity

N_WORDS = 32000
EDIM = 256
N_AGENTS = 131072
BAG = 8
N_BATCHES = 4096
N_ACTIONS = 16
NLAYERS = 3

N_CORES = 8
NSUB = 4                  # sub-shards per core
NSHARD = N_CORES * NSUB   # 32
NBG = 10                  # batch-groups per sub-shard
BG_BATCHES = 16
BG_AGENTS = 512
B_S = NBG * BG_BATCHES    # 160
A_S = NBG * BG_AGENTS     # 5120
G = 8
NG = A_S // G             # 640
NCHUNK = NG // 128        # 5
W2 = 2 * BG_BATCHES       # 32
P = 128
ZPAD_ROW = N_WORDS
DT = mybir.dt
AF = mybir.ActivationFunctionType
OP = mybir.AluOpType

_PROGRAM_CACHE = {}
_DEBUG = False


# ================================================================ host prep
def _pack_batches(counts):
    padded = ((counts + G - 1) // G) * G
    ngroups = NSHARD * NBG
    cap = np.full(ngroups, BG_AGENTS, dtype=np.int64)
    slots = np.full(ngroups, BG_BATCHES, dtype=np.int64)
    members = [[] for _ in range(ngroups)]
    for b in np.argsort(-padded, kind="stable"):
        ok = (cap >= padded[b]) & (slots > 0)
        if not ok.any():
            raise RuntimeError("bin packing failed")
        g = int(np.argmax(np.where(ok, cap, -1)))
        members[g].append(int(b))
        cap[g] -= padded[b]
        slots[g] -= 1
    return members, padded


def _build_host_inputs(x, batch_idx, batch_len, emb_table, W0, b0, W1, b1,
                       Wh, bh):
    bf16 = ml_dtypes.bfloat16
    x = np.asarray(x, dtype=np.int64)
    batch_idx = np.asarray(batch_idx, dtype=np.int64)
    batch_len64 = np.asarray(batch_len, dtype=np.float64)

    counts = np.bincount(batch_idx, minlength=N_BATCHES).astype(np.int64)
    starts = np.concatenate([[0], np.cumsum(counts)[:-1]])
    members, padded = _pack_batches(counts)

    # table rows pre-scaled by 1/8 (exact), bf16, with a zero pad row;
    # uint16 view for fast host-side take/transpose.
    tblu = np.concatenate(
        [np.asarray(emb_table, np.float32) / 8.0,
         np.zeros((1, EDIM), np.float32)], 0).astype(bf16).view(np.uint16)
    # dim-major word-plane slabs: slab_all[sh, t, k, p, a] =
    # table[x[a*BAG+k], t*128+p] / 8 (zeros for padded agent slots).
    # The device bag-sums the 8 k-planes via SWDGE accumulate-DMA (CCE ADD).
    slab_all = np.empty((NSHARD, 2, BAG, P, A_S), dtype=np.uint16)
    xsel_all = np.zeros((NSHARD, P, A_S), dtype=bf16)
    recip_all = np.zeros((NSHARD, P, A_S), dtype=bf16)
    s2_all = np.zeros((NSHARD, P, NCHUNK, W2), dtype=bf16)
    lenm1_all = np.zeros((NSHARD, 1, B_S), dtype=bf16)
    gcnt_all = np.full((NSHARD, 1, NBG // 2), BG_AGENTS * 2, dtype=np.int32)
    out_map = np.full((NSHARD, B_S), -1, dtype=np.int64)
    dbg_slots = []
    recip_f = (1.0 / (batch_len64 - 0.99999)).astype(np.float32)

    for sh in range(NSHARD):
        idx_flat = np.full((BAG, A_S), ZPAD_ROW, dtype=np.int64)
        a_of_slot = np.full(A_S, -1, dtype=np.int64)
        b_of_slot = np.full(A_S, -1, dtype=np.int64)
        grp_content = np.zeros(NBG, np.int64)
        for bg in range(NBG):
            pos = bg * BG_AGENTS
            for sl, b in enumerate(members[sh * NBG + bg]):
                lb = bg * BG_BATCHES + sl
                out_map[sh, lb] = b
                lenm1_all[sh, 0, lb] = np.float32(batch_len64[b] - 0.99999)
                n = int(counts[b])
                a_of_slot[pos:pos + n] = np.arange(starts[b], starts[b] + n)
                b_of_slot[pos:pos + n] = lb
                pos += int(padded[b])
            grp_content[bg] = pos - bg * BG_AGENTS
        slots = np.nonzero(a_of_slot >= 0)[0]
        ags = a_of_slot[slots]
        for k in range(BAG):
            idx_flat[k, slots] = x[ags * BAG + k]

        # pre-gather + transpose to dim-major word planes
        for k in range(BAG):
            gath = tblu[idx_flat[k]]               # [A_S, 256]
            slab_all[sh, :, k] = gath.T.reshape(2, P, A_S)

        lb_real = b_of_slot[slots]
        rec = recip_f[out_map[sh, lb_real]]
        recip_row = np.zeros(A_S, np.float32)
        recip_row[slots] = rec
        recip_all[sh] = np.broadcast_to(recip_row.astype(bf16), (P, A_S))

        j_of_slot = slots // BG_AGENTS
        w0_al = np.where(j_of_slot >= 8, 128, (j_of_slot // 4) * 64)
        r = lb_real - w0_al
        assert (r >= 0).all() and (r < P).all()
        xs = np.zeros((P, A_S), np.float32)
        xs[r, slots] = rec
        xsel_all[sh] = xs.astype(bf16)

        dbg_slots.append((a_of_slot.copy(), b_of_slot.copy()))
        g_b = b_of_slot[::G]
        for c in range(NCHUNK):
            gl = np.arange(P)
            gb = g_b[c * P + gl]
            v = gb >= 0
            w = gb[v] - W2 * c
            assert (w >= 0).all() and (w < W2).all()
            s2_all[sh][gl[v], c, w] = 1.0

    W0 = np.asarray(W0, np.float32)
    W1 = np.asarray(W1, np.float32)
    wl = np.stack([W0[:, :EDIM].T, W1[:, :EDIM].T])   # [layer, 256k, 256d]
    wr = np.stack([W0[:, EDIM:].T, W1[:, EDIM:].T])

    def tiles(w):  # [2,256,256] -> [128, 2(layer), 2(kt), 2(dt), 128]
        t = w.reshape(2, 2, P, 2, P).transpose(2, 0, 1, 3, 4)
        return np.ascontiguousarray(t).astype(bf16)

    host = {
        "wlT": tiles(wl),
        "wrTn": tiles(-wr),
        "wrT": tiles(wr),
        "bias": np.ascontiguousarray(
            np.stack([np.asarray(b0, np.float32), np.asarray(b1, np.float32)])
            .reshape(2, 2, P).transpose(1, 0, 2)[None]  # wrong axis order?
        ),
        "whT": np.ascontiguousarray(
            np.asarray(Wh, np.float32).T.reshape(2, P, N_ACTIONS)
            .transpose(1, 0, 2)).astype(bf16),
        "bh": np.asarray(bh, np.float32).reshape(1, N_ACTIONS).astype(bf16),
        "ones_b": np.ones((1, B_S), bf16),
    }
    # bias layout: [1, 2(layer), 2(dt), 128]
    bias = np.stack([np.asarray(b0, np.float32),
                     np.asarray(b1, np.float32)]).reshape(2, 2, P)
    host["bias"] = bias[None].astype(bf16)

    slab_bf = slab_all.view(bf16)
    per_core = []
    for core in range(N_CORES):
        s0 = core * NSUB
        m = dict(host)
        m["slabT"] = slab_bf[s0:s0 + NSUB]
        m["xsel"] = xsel_all[s0:s0 + NSUB]
        m["recipb"] = recip_all[s0:s0 + NSUB]
        m["s2"] = s2_all[s0:s0 + NSUB]
        m["lenm1"] = lenm1_all[s0:s0 + NSUB]
        per_core.append(m)
    return per_core, out_map, dbg_slots


# ============================================================ device program
def _build_program():
    nc = bacc.Bacc("TRN2")
    bf, f32, i32 = DT.bfloat16, DT.float32, DT.int32

    slab_d = nc.dram_tensor("slabT", [NSUB, 2, BAG, P, A_S], bf,
                            kind="ExternalInput")
    xsel_d = nc.dram_tensor("xsel", [NSUB, P, A_S], bf, kind="ExternalInput")
    recip_d = nc.dram_tensor("recipb", [NSUB, P, A_S], bf,
                             kind="ExternalInput")
    s2_d = nc.dram_tensor("s2", [NSUB, P, NCHUNK, W2], bf,
                          kind="ExternalInput")
    lenm1_d = nc.dram_tensor("lenm1", [NSUB, 1, B_S], bf,
                             kind="ExternalInput")
    wlT_d = nc.dram_tensor("wlT", [P, 2, 2, 2, P], bf, kind="ExternalInput")
    wrTn_d = nc.dram_tensor("wrTn", [P, 2, 2, 2, P], bf, kind="ExternalInput")
    wrT_d = nc.dram_tensor("wrT", [P, 2, 2, 2, P], bf, kind="ExternalInput")
    bias_d = nc.dram_tensor("bias", [1, 2, 2, P], bf, kind="ExternalInput")
    whT_d = nc.dram_tensor("whT", [P, 2, N_ACTIONS], bf,
                           kind="ExternalInput")
    bh_d = nc.dram_tensor("bh", [1, N_ACTIONS], bf, kind="ExternalInput")
    ones_d = nc.dram_tensor("ones_b", [1, B_S], bf, kind="ExternalInput")
    out_d = nc.dram_tensor("out", [NSUB, B_S, N_ACTIONS], f32,
                           kind="ExternalOutput")
    if _DEBUG:
        dbg_emb0 = nc.dram_tensor("dbg_emb0", [2, P, A_S], bf,
                                  kind="ExternalOutput")
        dbg_emb1 = nc.dram_tensor("dbg_emb1", [2, P, A_S], bf,
                                  kind="ExternalOutput")
        dbg_mT = nc.dram_tensor("dbg_mT", [P, 512], f32,
                                kind="ExternalOutput")
        dbg_x2 = nc.dram_tensor("dbg_x2", [2, P, A_S], bf,
                                kind="ExternalOutput")
        dbg_r = nc.dram_tensor("dbg_r", [P, 2 * B_S], f32,
                               kind="ExternalOutput")
        dbg_h = nc.dram_tensor("dbg_h", [N_ACTIONS, B_S], f32,
                               kind="ExternalOutput")

    with tile.TileContext(nc) as tc, ExitStack() as ctx:
        consts = ctx.enter_context(tc.tile_pool(name="consts", bufs=1))
        wpool = ctx.enter_context(tc.tile_pool(name="wpool", bufs=1))
        gpool = ctx.enter_context(tc.tile_pool(name="gath", bufs=2))
        epool = ctx.enter_context(tc.tile_pool(name="emb", bufs=2))
        e1pool = ctx.enter_context(tc.tile_pool(name="emb1", bufs=2))
        e2pool = ctx.enter_context(tc.tile_pool(name="emb2", bufs=1))
        xpool = ctx.enter_context(tc.tile_pool(name="x2p", bufs=1))
        spool = ctx.enter_context(tc.tile_pool(name="small", bufs=2))
        main_ps = ctx.enter_context(
            tc.tile_pool(name="mps", bufs=4, space="PSUM"))
        tp_ps = ctx.enter_context(
            tc.tile_pool(name="tps", bufs=2, space="PSUM"))
        sm_ps = ctx.enter_context(
            tc.tile_pool(name="sps", bufs=2, space="PSUM"))

        ident = consts.tile([P, P], f32, tag="ident", name="ident")
        make_identity(nc, ident[:])

        wlT = wpool.tile([P, 2, 2, 2, P], bf, tag="wlT", name="wlT")
        nc.sync.dma_start(wlT[:], wlT_d[:])
        wrTn = wpool.tile([P, 2, 2, 2, P], bf, tag="wrTn", name="wrTn")
        nc.sync.dma_start(wrTn[:], wrTn_d[:])
        wrT = wpool.tile([P, 2, 2, 2, P], bf, tag="wrT", name="wrT")
        nc.sync.dma_start(wrT[:], wrT_d[:])
        bias_sb = wpool.tile([1, 2, 2, P], bf, tag="bias", name="bias")
        nc.sync.dma_start(bias_sb[:], bias_d[:])
        whT = wpool.tile([P, 2, N_ACTIONS], bf, tag="whT", name="whT")
        nc.sync.dma_start(whT[:], whT_d[:])
        bh_sb = wpool.tile([1, N_ACTIONS], bf, tag="bh", name="bh")
        nc.sync.dma_start(bh_sb[:], bh_d[:])
        ones_sb = wpool.tile([1, B_S], bf, tag="ones", name="ones")
        nc.sync.dma_start(ones_sb[:], ones_d[:])

        def tpack(src_aps, dst_ap):
            """PE-transpose [p<=128, w<=128] fp32 APs into one psum bank,
            then one ACT copy (w/ cast) into dst_ap (columns concatenated).
            Each src must have 128 partitions."""
            ps = tp_ps.tile([P, 512], f32, tag="tpack", name="tpack")
            col = 0
            for a in src_aps:
                w = a.shape[-1]
                nc.tensor.transpose(ps[:, col:col + w], a, ident[:])
                col += w
            nc.scalar.activation(dst_ap, ps[:, :col], AF.Copy)

        for s in range(NSUB):
            # ---------------- Phase A: stream pre-gathered slabs + bag-sum
            xsel = gpool.tile([P, A_S], bf, tag="xsel", name="xsel", bufs=1)
            nc.sync.dma_start(xsel[:], xsel_d[s])
            recipb = gpool.tile([P, A_S], bf, tag="recipb", name="recipb", bufs=1)
            nc.sync.dma_start(recipb[:], recip_d[s])
            s2_sb = gpool.tile([P, NCHUNK, W2], bf, tag="s2", name="s2")
            nc.sync.dma_start(s2_sb[:], s2_d[s])
            lenm1 = gpool.tile([1, B_S], bf, tag="lenm1", name="lenm1")
            nc.sync.dma_start(lenm1[:], lenm1_d[s])

            emb = [None] * NLAYERS
            emb[0] = [epool.tile([P, A_S], bf, tag=f"emb0_{t}", name=f"emb0_{t}")
                      for t in range(2)]
            emb[1] = [e1pool.tile([P, A_S], bf, tag=f"emb1_{t}", name=f"emb1_{t}")
                      for t in range(2)]
            emb[2] = [e2pool.tile([P, A_S], bf, tag=f"emb2_{t}", name=f"emb2_{t}")
                      for t in range(2)]

            # bag-sum of the 8 word planes, split across units: plane 0
            # lands via plain HWDGE, plane 1 accumulates inside the SDMA
            # engines (CCE ADD; descriptors >2048 elements corrupt, so
            # 2048-wide chunks; same-address RMW needs sem-ordered ops),
            # planes 2-7 go through a DVE pairwise add tree (2x bf16 uop;
            # tensor_reduce only has the 1x uop).
            CW = A_S // 4
            for t in range(2):
                nc.sync.dma_start(emb[0][t][:], slab_d[s, t, 0])
                for c0 in range(0, A_S, 2048):
                    c1 = min(c0 + 2048, A_S)
                    nc.gpsimd.dma_start(emb[0][t][:, c0:c1],
                                        slab_d[s, t, 1, :, c0:c1],
                                        accum_op=OP.add)
                for c in range(4):
                    st = gpool.tile([P, 6, CW], bf, tag="stage",
                                    name="stage", bufs=2)
                    cs = slice(c * CW, (c + 1) * CW)
                    for k in range(6):
                        nc.sync.dma_start(st[:, k, :],
                                          slab_d[s, t, 2 + k, :, cs])
                    for (d, a, b) in ((0, 0, 1), (2, 2, 3), (4, 4, 5),
                                     (0, 0, 2), (0, 0, 4)):
                        nc.vector.tensor_tensor(out=st[:, d, :],
                                                in0=st[:, a, :],
                                                in1=st[:, b, :], op=OP.add)
                    nc.vector.tensor_tensor(out=emb[0][t][:, cs],
                                            in0=emb[0][t][:, cs],
                                            in1=st[:, 0, :], op=OP.add)

            if _DEBUG and s == 0:
                for t in range(2):
                    nc.sync.dma_start(dbg_emb0[t], emb[0][t][:])

            # ---------------- helpers
            def segsum(src):
                """src = [t0, t1] bf16 [P, A_S] -> m^T psum [P, 512] f32:
                cols 0:256 = batches 0:128, cols 256:512 = batches 128:160
                (on partitions 0:32)."""
                grp = [spool.tile([P, NG], f32, tag=f"grp{t}", name=f"grp{t}",
                                  bufs=1)
                       for t in range(2)]
                for t in range(2):
                    nc.vector.tensor_reduce(
                        grp[t][:],
                        src[t][:].rearrange("p (g e) -> p g e", e=G),
                        axis=mybir.AxisListType.X, op=OP.add)
                gt = spool.tile([P, NCHUNK * EDIM], bf, tag="gt", name="gt")
                for c in range(NCHUNK):
                    tpack([grp[t][:, c * P:(c + 1) * P] for t in range(2)],
                          gt[:, c * EDIM:(c + 1) * EDIM])
                m_ps = sm_ps.tile([P, 512], f32, tag="sps", name="sps")
                for c in range(NCHUNK):
                    r0 = W2 * c if c < 4 else 0
                    dst = (m_ps[r0:r0 + W2, 0:EDIM] if c < 4
                           else m_ps[0:W2, EDIM:2 * EDIM])
                    nc.tensor.matmul(dst, lhsT=s2_sb[:, c, :],
                                     rhs=gt[:, c * EDIM:(c + 1) * EDIM],
                                     start=True, stop=True,
                                     skip_group_check=True,
                                     tile_position=(0, r0))
                return m_ps

            def m_to_sbuf(m_ps):
                mT = spool.tile([P, 512], f32, tag="mT", name="mT")
                nc.scalar.activation(mT[:, 0:EDIM], m_ps[:, 0:EDIM], AF.Copy)
                nc.scalar.activation(mT[0:W2, EDIM:2 * EDIM],
                                     m_ps[0:W2, EDIM:2 * EDIM], AF.Copy)
                return mT

            def m_dimmajor(mT_sb):
                """m^T sbuf -> mdm bf16 [P, 2(dt), B_S] (dim-major m)."""
                ps = sm_ps.tile([P, 512], f32, tag="sps", name="sps")
                for t in range(2):
                    nc.tensor.transpose(ps[:, t * B_S:t * B_S + P],
                                        mT_sb[:, t * P:(t + 1) * P],
                                        ident[:])
                    nc.tensor.transpose(
                        ps[:, t * B_S + P:t * B_S + B_S],
                        mT_sb[0:W2, EDIM + t * P:EDIM + (t + 1) * P],
                        ident[0:W2, 0:W2])
                out = spool.tile([P, 2 * B_S], bf, tag="mdm", name="mdm")
                nc.scalar.activation(out[:], ps[:, 0:2 * B_S], AF.Copy)
                return out

            # ---------------- layers 0, 1
            for i in range(2):
                x2 = [xpool.tile([P, A_S], bf, tag=f"x2_{t}", name=f"x2_{t}")
                      for t in range(2)]
                for t in range(2):
                    nc.vector.tensor_tensor(out=x2[t][:], in0=emb[i][t][:],
                                            in1=recipb[:], op=OP.mult)
                m_ps = segsum(emb[i])
                mT_sb_dbg = m_to_sbuf(m_ps)
                if _DEBUG and s == 0 and i == 0:
                    for t in range(2):
                        nc.sync.dma_start(dbg_x2[t], x2[t][:])
                    nc.sync.dma_start(dbg_mT[:], mT_sb_dbg[:])
                mdm = m_dimmajor(mT_sb_dbg)
                r_ps = sm_ps.tile([P, 512], f32, tag="sps", name="sps")
                for dt in range(2):
                    sl = r_ps[:, dt * B_S:(dt + 1) * B_S]
                    for kt in range(2):
                        nc.tensor.matmul(
                            sl, lhsT=wrT[:, i, kt, dt, :],
                            rhs=mdm[:, kt * B_S:(kt + 1) * B_S],
                            start=(kt == 0), stop=False)
                    nc.tensor.matmul(sl, lhsT=bias_sb[:, i, dt, :],
                                     rhs=lenm1[:], start=False, stop=True)
                r_sb = spool.tile([P, 2 * B_S], f32, tag="r_sb", name="r_sb")
                nc.scalar.activation(r_sb[:], r_ps[:, 0:2 * B_S], AF.Copy)
                if _DEBUG and s == 0 and i == 0:
                    nc.sync.dma_start(dbg_r[:], r_sb[:])
                # R^T at alignments 0 / 64 / 128 -> rt [P, 3, 256] bf16
                rt = spool.tile([P, 3, EDIM], bf, tag="rt", name="rt")
                nc.vector.memset(rt[:], 0.0)
                rt_ps = sm_ps.tile([P, 512], f32, tag="sps", name="sps")
                for dt in range(2):
                    nc.tensor.transpose(
                        rt_ps[:, dt * P:(dt + 1) * P],
                        r_sb[:, dt * B_S:dt * B_S + P], ident[:])
                nc.scalar.activation(rt[:, 0, :], rt_ps[:, 0:EDIM], AF.Copy)
                rt_ps2 = sm_ps.tile([P, 512], f32, tag="sps", name="sps")
                for dt in range(2):
                    nc.tensor.transpose(
                        rt_ps2[0:96, dt * P:(dt + 1) * P],
                        r_sb[:, dt * B_S + 64:dt * B_S + B_S], ident[:])
                    nc.tensor.transpose(
                        rt_ps2[0:W2, EDIM + dt * P:EDIM + dt * P + P],
                        r_sb[:, dt * B_S + P:dt * B_S + B_S], ident[:])
                nc.scalar.activation(rt[0:96, 1, :], rt_ps2[0:96, 0:EDIM],
                                     AF.Copy)
                nc.scalar.activation(rt[0:W2, 2, :],
                                     rt_ps2[0:W2, EDIM:2 * EDIM], AF.Copy)
                # main matmuls: weights-outer over 4-block chunks so
                # consecutive matmuls share the stationary operand
                for j0 in range(0, NBG, 4):
                    jn = min(4, NBG - j0)
                    jss = [slice((j0 + jj) * BG_AGENTS,
                                 (j0 + jj + 1) * BG_AGENTS)
                           for jj in range(jn)]
                    al = min(j0 // 4, 2)
                    for dt in range(2):
                        pss = [main_ps.tile([P, BG_AGENTS], f32, tag="main",
                                            name="main")
                               for _ in range(jn)]
                        for kt in range(2):
                            for jj in range(jn):
                                nc.tensor.matmul(
                                    pss[jj][:], lhsT=wlT[:, i, kt, dt, :],
                                    rhs=emb[i][kt][:, jss[jj]],
                                    start=(kt == 0), stop=False)
                        for kt in range(2):
                            for jj in range(jn):
                                nc.tensor.matmul(
                                    pss[jj][:], lhsT=wrTn[:, i, kt, dt, :],
                                    rhs=x2[kt][:, jss[jj]],
                                    start=False, stop=False)
                        for jj in range(jn):
                            nc.tensor.matmul(
                                pss[jj][:],
                                lhsT=rt[:, al, dt * P:(dt + 1) * P],
                                rhs=xsel[:, jss[jj]],
                                start=False, stop=True)
                        for jj in range(jn):
                            nc.scalar.activation(emb[i + 1][dt][:, jss[jj]],
                                                 pss[jj][:], AF.Relu)
                if _DEBUG and s == 0 and i == 0:
                    for t in range(2):
                        nc.sync.dma_start(dbg_emb1[t], emb[1][t][:])

            # ---------------- final segsum + head + log_softmax
            m_ps = segsum(emb[2])
            mdm = m_dimmajor(m_to_sbuf(m_ps))
            h_ps = sm_ps.tile([P, 512], f32, tag="sps", name="sps")
            hsl = h_ps[0:N_ACTIONS, 0:B_S]
            for kt in range(2):
                nc.tensor.matmul(hsl, lhsT=whT[:, kt, :],
                                 rhs=mdm[:, kt * B_S:(kt + 1) * B_S],
                                 start=(kt == 0), stop=False)
            nc.tensor.matmul(hsl, lhsT=bh_sb[:], rhs=ones_sb[:],
                             start=False, stop=True)
            h_sb = spool.tile([N_ACTIONS, B_S], f32, tag="h_sb", name="h_sb")
            nc.scalar.activation(h_sb[:], hsl, AF.Copy)
            if _DEBUG and s == 0:
                nc.sync.dma_start(dbg_h[:], h_sb[:])
            lg_ps = sm_ps.tile([P, 512], f32, tag="sps", name="sps")
            nc.tensor.transpose(lg_ps[:, 0:N_ACTIONS], h_sb[:, 0:P],
                                ident[0:N_ACTIONS, 0:N_ACTIONS])
            nc.tensor.transpose(lg_ps[0:W2, N_ACTIONS:2 * N_ACTIONS],
                                h_sb[:, P:B_S],
                                ident[0:N_ACTIONS, 0:N_ACTIONS])
            lg = spool.tile([P, 2 * N_ACTIONS], f32, tag="lg_sb", name="lg_sb")
            nc.scalar.activation(lg[:, 0:N_ACTIONS], lg_ps[:, 0:N_ACTIONS],
                                 AF.Copy)
            nc.scalar.activation(lg[0:W2, N_ACTIONS:2 * N_ACTIONS],
                                 lg_ps[0:W2, N_ACTIONS:2 * N_ACTIONS],
                                 AF.Copy)
            for part in range(2):
                rows = P if part == 0 else B_S - P
                src = lg[0:rows, part * N_ACTIONS:(part + 1) * N_ACTIONS]
                mx = spool.tile([P, 1], f32, tag="mx", name="mx")
                nc.vector.tensor_reduce(mx[0:rows, :], src,
                                        axis=mybir.AxisListType.X,
                                        op=OP.max)
                shv = spool.tile([P, N_ACTIONS], f32, tag="shift", name="shift")
                nc.vector.tensor_tensor(
                    out=shv[0:rows, :], in0=src,
                    in1=mx[0:rows, :].to_broadcast([rows, N_ACTIONS]),
                    op=OP.subtract)
                ex = spool.tile([P, N_ACTIONS], f32, tag="ex", name="ex")
                se = spool.tile([P, 1], f32, tag="se", name="se")
                nc.scalar.activation(ex[0:rows, :], shv[0:rows, :], AF.Exp,
                                     accum_out=se[0:rows, :])
                lse = spool.tile([P, 1], f32, tag="lse", name="lse")
                nc.scalar.activation(lse[0:rows, :], se[0:rows, :], AF.Ln)
                res = spool.tile([P, N_ACTIONS], f32, tag="res", name="res")
                nc.vector.tensor_tensor(
                    out=res[0:rows, :], in0=shv[0:rows, :],
                    in1=lse[0:rows, :].to_broadcast([rows, N_ACTIONS]),
                    op=OP.subtract)
                nc.sync.dma_start(out_d[s, part * P:part * P + rows, :],
                                  res[0:rows, :])
    nc.compile()
    return nc


# ================================================================== kernel
def kernel(**inputs):
    per_core, out_map, _ = _build_host_inputs(
        inputs["x"], inputs["batch_idx"], inputs["batch_len"],
        inputs["emb_table"], inputs["W0"], inputs["b0"], inputs["W1"],
        inputs["b1"], inputs["Wh"], inputs["bh"])

    if "prog" not in _PROGRAM_CACHE:
        _PROGRAM_CACHE["prog"] = _build_program()
    nc = _PROGRAM_CACHE["prog"]

    from concourse.bass_utils import run_bass_kernel_spmd
    res = run_bass_kernel_spmd(nc, per_core, core_ids=list(range(N_CORES)))

    logp = np.zeros((N_BATCHES, N_ACTIONS), np.float32)
    for core in range(N_CORES):
        out = np.asarray(res.results[core]["out"], np.float32)
        for s in range(NSUB):
            sh = core * NSUB + s
            v = out_map[sh] >= 0
            logp[out_map[sh][v]] = out[s][v]
    return (logp,)



# revision 21
# speedup vs baseline: 1.4360x; 1.0315x over previous
"""Trainium2 Bass kernel for nn_Commnet (gnn_message_passing).

kernel(**inputs) takes FULL unsharded numpy inputs, returns (logp [4096,16],)
computed across 8 NeuronCores (SPMD single program; per-core structure is
carried entirely in input tensors).

Sharding: batches bin-packed into 32 sub-shards (4/core); each sub-shard =
10 batch-groups of 16 batch slots padded to exactly 512 agent slots, so every
512-agent matmul block has a static 16-batch selector window -> the program
is identical on all cores.

EmbeddingBag(mean): table cast to bf16 pre-scaled by 1/8 on host (exact);
the host pre-permutes the per-agent word rows into dim-major slabs
slabT[t][dim, agent*8+word] (a pure row gather/layout, no arithmetic) so the
device streams them with large contiguous HWDGE DMAs at line rate; the
bag-sum is a DVE group-reduce(8) along the free dim, directly producing the
dim-major agent embeddings (no PE transposes, no SWDGE descriptor storm).

Middle layers: emb' = relu(W_l@emb - W_r@(emb*recip) + R'@sel) where
R' = W_r@m + b (x) (len-0.99999) folds expansion+bias; sel is a banded
one-hot*recip selector (1 extra k-tile per block). m = segment sum via DVE
group-sum(8) -> PE transpose -> banded matmuls into disjoint PSUM windows.
Padded agents stay exactly 0 through all layers.
"""

from contextlib import ExitStack

import numpy as np
import ml_dtypes

import concourse.bass as bass
import concourse.bacc as bacc
import concourse.tile as tile
from concourse import mybir
from concourse.masks import make_identity

N_WORDS = 32000
EDIM = 256
N_AGENTS = 131072
BAG = 8
N_BATCHES = 4096
N_ACTIONS = 16
NLAYERS = 3

N_CORES = 8
NSUB = 4                  # sub-shards per core
NSHARD = N_CORES * NSUB   # 32
NBG = 10                  # batch-groups per sub-shard
BG_BATCHES = 16
BG_AGENTS = 512
B_S = NBG * BG_BATCHES    # 160
A_S = NBG * BG_AGENTS     # 5120
G = 8
NG = A_S // G             # 640
NCHUNK = NG // 128        # 5
W2 = 2 * BG_BATCHES       # 32
P = 128
ZPAD_ROW = N_WORDS
DT = mybir.dt
AF = mybir.ActivationFunctionType
OP = mybir.AluOpType

_PROGRAM_CACHE = {}
_DEBUG = False


# ================================================================ host prep
def _pack_batches(counts):
    padded = ((counts + G - 1) // G) * G
    ngroups = NSHARD * NBG
    cap = np.full(ngroups, BG_AGENTS, dtype=np.int64)
    slots = np.full(ngroups, BG_BATCHES, dtype=np.int64)
    members = [[] for _ in range(ngroups)]
    for b in np.argsort(-padded, kind="stable"):
        ok = (cap >= padded[b]) & (slots > 0)
        if not ok.any():
            raise RuntimeError("bin packing failed")
        g = int(np.argmax(np.where(ok, cap, -1)))
        members[g].append(int(b))
        cap[g] -= padded[b]
        slots[g] -= 1
    return members, padded


def _build_host_inputs(x, batch_idx, batch_len, emb_table, W0, b0, W1, b1,
                       Wh, bh):
    bf16 = ml_dtypes.bfloat16
    x = np.asarray(x, dtype=np.int64)
    batch_idx = np.asarray(batch_idx, dtype=np.int64)
    batch_len64 = np.asarray(batch_len, dtype=np.float64)

    counts = np.bincount(batch_idx, minlength=N_BATCHES).astype(np.int64)
    starts = np.concatenate([[0], np.cumsum(counts)[:-1]])
    members, padded = _pack_batches(counts)

    # table rows pre-scaled by 1/8 (exact), bf16, with a zero pad row;
    # uint16 view for fast host-side take/transpose.
    tblu = np.concatenate(
        [np.asarray(emb_table, np.float32) / 8.0,
         np.zeros((1, EDIM), np.float32)], 0).astype(bf16).view(np.uint16)
    # dim-major word-plane slabs: slab_all[sh, t, k, p, a] =
    # table[x[a*BAG+k], t*128+p] / 8 (zeros for padded agent slots).
    # The device bag-sums the 8 k-planes via SWDGE accumulate-DMA (CCE ADD).
    slab_all = np.empty((NSHARD, 2, BAG, P, A_S), dtype=np.uint16)
    xsel_all = np.zeros((NSHARD, P, A_S), dtype=bf16)
    recip_all = np.zeros((NSHARD, P, A_S), dtype=bf16)
    s2_all = np.zeros((NSHARD, P, NCHUNK, W2), dtype=bf16)
    lenm1_all = np.zeros((NSHARD, 1, B_S), dtype=bf16)
    gcnt_all = np.full((NSHARD, 1, NBG // 2), BG_AGENTS * 2, dtype=np.int32)
    out_map = np.full((NSHARD, B_S), -1, dtype=np.int64)
    dbg_slots = []
    recip_f = (1.0 / (batch_len64 - 0.99999)).astype(np.float32)

    for sh in range(NSHARD):
        idx_flat = np.full((BAG, A_S), ZPAD_ROW, dtype=np.int64)
        a_of_slot = np.full(A_S, -1, dtype=np.int64)
        b_of_slot = np.full(A_S, -1, dtype=np.int64)
        grp_content = np.zeros(NBG, np.int64)
        for bg in range(NBG):
            pos = bg * BG_AGENTS
            for sl, b in enumerate(members[sh * NBG + bg]):
                lb = bg * BG_BATCHES + sl
                out_map[sh, lb] = b
                lenm1_all[sh, 0, lb] = np.float32(batch_len64[b] - 0.99999)
                n = int(counts[b])
                a_of_slot[pos:pos + n] = np.arange(starts[b], starts[b] + n)
                b_of_slot[pos:pos + n] = lb
                pos += int(padded[b])
            grp_content[bg] = pos - bg * BG_AGENTS
        slots = np.nonzero(a_of_slot >= 0)[0]
        ags = a_of_slot[slots]
        for k in range(BAG):
            idx_flat[k, slots] = x[ags * BAG + k]

        # pre-gather + transpose to dim-major word planes
        for k in range(BAG):
            gath = tblu[idx_flat[k]]               # [A_S, 256]
            slab_all[sh, :, k] = gath.T.reshape(2, P, A_S)

        lb_real = b_of_slot[slots]
        rec = recip_f[out_map[sh, lb_real]]
        recip_row = np.zeros(A_S, np.float32)
        recip_row[slots] = rec
        recip_all[sh] = np.broadcast_to(recip_row.astype(bf16), (P, A_S))

        j_of_slot = slots // BG_AGENTS
        w0_al = np.where(j_of_slot >= 8, 128, (j_of_slot // 4) * 64)
        r = lb_real - w0_al
        assert (r >= 0).all() and (r < P).all()
        xs = np.zeros((P, A_S), np.float32)
        xs[r, slots] = rec
        xsel_all[sh] = xs.astype(bf16)

        dbg_slots.append((a_of_slot.copy(), b_of_slot.copy()))
        g_b = b_of_slot[::G]
        for c in range(NCHUNK):
            gl = np.arange(P)
            gb = g_b[c * P + gl]
            v = gb >= 0
            w = gb[v] - W2 * c
            assert (w >= 0).all() and (w < W2).all()
            s2_all[sh][gl[v], c, w] = 1.0

    W0 = np.asarray(W0, np.float32)
    W1 = np.asarray(W1, np.float32)
    wl = np.stack([W0[:, :EDIM].T, W1[:, :EDIM].T])   # [layer, 256k, 256d]
    wr = np.stack([W0[:, EDIM:].T, W1[:, EDIM:].T])

    def tiles(w):  # [2,256,256] -> [128, 2(layer), 2(kt), 2(dt), 128]
        t = w.reshape(2, 2, P, 2, P).transpose(2, 0, 1, 3, 4)
        return np.ascontiguousarray(t).astype(bf16)

    host = {
        "wlT": tiles(wl),
        "wrTn": tiles(-wr),
        "wrT": tiles(wr),
        "bias": np.ascontiguousarray(
            np.stack([np.asarray(b0, np.float32), np.asarray(b1, np.float32)])
            .reshape(2, 2, P).transpose(1, 0, 2)[None]  # wrong axis order?
        ),
        "whT": np.ascontiguousarray(
            np.asarray(Wh, np.float32).T.reshape(2, P, N_ACTIONS)
            .transpose(1, 0, 2)).astype(bf16),
        "bh": np.asarray(bh, np.float32).reshape(1, N_ACTIONS).astype(bf16),
        "ones_b": np.ones((1, B_S), bf16),
    }
    # bias layout: [1, 2(layer), 2(dt), 128]
    bias = np.stack([np.asarray(b0, np.float32),
                     np.asarray(b1, np.float32)]).reshape(2, 2, P)
    host["bias"] = bias[None].astype(bf16)

    slab_bf = slab_all.view(bf16)
    per_core = []
    for core in range(N_CORES):
        s0 = core * NSUB
        m = dict(host)
        m["slabT"] = slab_bf[s0:s0 + NSUB]
        m["xsel"] = xsel_all[s0:s0 + NSUB]
        m["recipb"] = recip_all[s0:s0 + NSUB]
        m["s2"] = s2_all[s0:s0 + NSUB]
        m["lenm1"] = lenm1_all[s0:s0 + NSUB]
        per_core.append(m)
    return per_core, out_map, dbg_slots


# ============================================================ device program
def _build_program():
    nc = bacc.Bacc("TRN2")
    bf, f32, i32 = DT.bfloat16, DT.float32, DT.int32

    slab_d = nc.dram_tensor("slabT", [NSUB, 2, BAG, P, A_S], bf,
                            kind="ExternalInput")
    xsel_d = nc.dram_tensor("xsel", [NSUB, P, A_S], bf, kind="ExternalInput")
    recip_d = nc.dram_tensor("recipb", [NSUB, P, A_S], bf,
                             kind="ExternalInput")
    s2_d = nc.dram_tensor("s2", [NSUB, P, NCHUNK, W2], bf,
                          kind="ExternalInput")
    lenm1_d = nc.dram_tensor("lenm1", [NSUB, 1, B_S], bf,
                             kind="ExternalInput")
    wlT_d = nc.dram_tensor("wlT", [P, 2, 2, 2, P], bf, kind="ExternalInput")
    wrTn_d = nc.dram_tensor("wrTn", [P, 2, 2, 2, P], bf, kind="ExternalInput")
    wrT_d = nc.dram_tensor("wrT", [P, 2, 2, 2, P], bf, kind="ExternalInput")
    bias_d = nc.dram_tensor("bias", [1, 2, 2, P], bf, kind="ExternalInput")
    whT_d = nc.dram_tensor("whT", [P, 2, N_ACTIONS], bf,
                           kind="ExternalInput")
    bh_d = nc.dram_tensor("bh", [1, N_ACTIONS], bf, kind="ExternalInput")
    ones_d = nc.dram_tensor("ones_b", [1, B_S], bf, kind="ExternalInput")
    out_d = nc.dram_tensor("out", [NSUB, B_S, N_ACTIONS], f32,
                           kind="ExternalOutput")
    if _DEBUG:
        dbg_emb0 = nc.dram_tensor("dbg_emb0", [2, P, A_S], bf,
                                  kind="ExternalOutput")
        dbg_emb1 = nc.dram_tensor("dbg_emb1", [2, P, A_S], bf,
                                  kind="ExternalOutput")
        dbg_mT = nc.dram_tensor("dbg_mT", [P, 512], f32,
                                kind="ExternalOutput")
        dbg_x2 = nc.dram_tensor("dbg_x2", [2, P, A_S], bf,
                                kind="ExternalOutput")
        dbg_r = nc.dram_tensor("dbg_r", [P, 2 * B_S], f32,
                               kind="ExternalOutput")
        dbg_h = nc.dram_tensor("dbg_h", [N_ACTIONS, B_S], f32,
                               kind="ExternalOutput")

    with tile.TileContext(nc) as tc, ExitStack() as ctx:
        consts = ctx.enter_context(tc.tile_pool(name="consts", bufs=1))
        wpool = ctx.enter_context(tc.tile_pool(name="wpool", bufs=1))
        gpool = ctx.enter_context(tc.tile_pool(name="gath", bufs=2))
        epool = ctx.enter_context(tc.tile_pool(name="emb", bufs=2))
        e1pool = ctx.enter_context(tc.tile_pool(name="emb1", bufs=2))
        e2pool = ctx.enter_context(tc.tile_pool(name="emb2", bufs=1))
        xpool = ctx.enter_context(tc.tile_pool(name="x2p", bufs=1))
        spool = ctx.enter_context(tc.tile_pool(name="small", bufs=2))
        main_ps = ctx.enter_context(
            tc.tile_pool(name="mps", bufs=4, space="PSUM"))
        tp_ps = ctx.enter_context(
            tc.tile_pool(name="tps", bufs=2, space="PSUM"))
        sm_ps = ctx.enter_context(
            tc.tile_pool(name="sps", bufs=2, space="PSUM"))

        ident = consts.tile([P, P], f32, tag="ident", name="ident")
        make_identity(nc, ident[:])

        wlT = wpool.tile([P, 2, 2, 2, P], bf, tag="wlT", name="wlT")
        nc.sync.dma_start(wlT[:], wlT_d[:])
        wrTn = wpool.tile([P, 2, 2, 2, P], bf, tag="wrTn", name="wrTn")
        nc.sync.dma_start(wrTn[:], wrTn_d[:])
        wrT = wpool.tile([P, 2, 2, 2, P], bf, tag="wrT", name="wrT")
        nc.sync.dma_start(wrT[:], wrT_d[:])
        bias_sb = wpool.tile([1, 2, 2, P], bf, tag="bias", name="bias")
        nc.sync.dma_start(bias_sb[:], bias_d[:])
        whT = wpool.tile([P, 2, N_ACTIONS], bf, tag="whT", name="whT")
        nc.sync.dma_start(whT[:], whT_d[:])
        bh_sb = wpool.tile([1, N_ACTIONS], bf, tag="bh", name="bh")
        nc.sync.dma_start(bh_sb[:], bh_d[:])
        ones_sb = wpool.tile([1, B_S], bf, tag="ones", name="ones")
        nc.sync.dma_start(ones_sb[:], ones_d[:])

        def tpack(src_aps, dst_ap):
            """PE-transpose [p<=128, w<=128] fp32 APs into one psum bank,
            then one ACT copy (w/ cast) into dst_ap (columns concatenated).
            Each src must have 128 partitions."""
            ps = tp_ps.tile([P, 512], f32, tag="tpack", name="tpack")
            col = 0
            for a in src_aps:
                w = a.shape[-1]
                nc.tensor.transpose(ps[:, col:col + w], a, ident[:])
                col += w
            nc.scalar.activation(dst_ap, ps[:, :col], AF.Copy)

        for s in range(NSUB):
            # ---------------- Phase A: stream pre-gathered slabs + bag-sum
            xsel = gpool.tile([P, A_S], bf, tag="xsel", name="xsel", bufs=1)
            nc.sync.dma_start(xsel[:], xsel_d[s])
            recipb = gpool.tile([P, A_S], bf, tag="recipb", name="recipb", bufs=1)
            nc.sync.dma_start(recipb[:], recip_d[s])
            s2_sb = gpool.tile([P, NCHUNK, W2], bf, tag="s2", name="s2")
            nc.sync.dma_start(s2_sb[:], s2_d[s])
            lenm1 = gpool.tile([1, B_S], bf, tag="lenm1", name="lenm1")
            nc.sync.dma_start(lenm1[:], lenm1_d[s])

            emb = [None] * NLAYERS
            emb[0] = [epool.tile([P, A_S], bf, tag=f"emb0_{t}", name=f"emb0_{t}")
                      for t in range(2)]
            emb[1] = [e1pool.tile([P, A_S], bf, tag=f"emb1_{t}", name=f"emb1_{t}")
                      for t in range(2)]
            emb[2] = [e2pool.tile([P, A_S], bf, tag=f"emb2_{t}", name=f"emb2_{t}")
                      for t in range(2)]

            # bag-sum of the 8 word planes, split across units: plane 0
            # lands via plain HWDGE, plane 1 accumulates inside the SDMA
            # engines (CCE ADD; descriptors >2048 elements corrupt, so
            # 2048-wide chunks; same-address RMW needs sem-ordered ops),
            # planes 2-7 go through a DVE pairwise add tree (2x bf16 uop;
            # tensor_reduce only has the 1x uop).
            CW = A_S // 4
            for t in range(2):
                nc.sync.dma_start(emb[0][t][:], slab_d[s, t, 0])
                for c0 in range(0, A_S, 2048):
                    c1 = min(c0 + 2048, A_S)
                    nc.gpsimd.dma_start(emb[0][t][:, c0:c1],
                                        slab_d[s, t, 1, :, c0:c1],
                                        accum_op=OP.add)
                for c in range(4):
                    st = gpool.tile([P, 6, CW], bf, tag="stage",
                                    name="stage", bufs=2)
                    cs = slice(c * CW, (c + 1) * CW)
                    for k in range(6):
                        nc.sync.dma_start(st[:, k, :],
                                          slab_d[s, t, 2 + k, :, cs])
                    for (d, a, b) in ((0, 0, 1), (2, 2, 3), (4, 4, 5),
                                     (0, 0, 2), (0, 0, 4)):
                        nc.vector.tensor_tensor(out=st[:, d, :],
                                                in0=st[:, a, :],
                                                in1=st[:, b, :], op=OP.add)
                    nc.vector.tensor_tensor(out=emb[0][t][:, cs],
                                            in0=emb[0][t][:, cs],
                                            in1=st[:, 0, :], op=OP.add)

            if _DEBUG and s == 0:
                for t in range(2):
                    nc.sync.dma_start(dbg_emb0[t], emb[0][t][:])

            # ---------------- helpers
            def segsum(src):
                """src = [t0, t1] bf16 [P, A_S] -> m^T psum [P, 512] f32:
                cols 0:256 = batches 0:128, cols 256:512 = batches 128:160
                (on partitions 0:32)."""
                grp = [spool.tile([P, NG], f32, tag=f"grp{t}", name=f"grp{t}",
                                  bufs=1)
                       for t in range(2)]
                gpb = BG_AGENTS // G
                for t in range(2):
                    for j in range(NBG):
                        nc.vector.tensor_reduce(
                            grp[t][:, j * gpb:(j + 1) * gpb],
                            src[t][:, j * BG_AGENTS:(j + 1) * BG_AGENTS]
                            .rearrange("p (g e) -> p g e", e=G),
                            axis=mybir.AxisListType.X, op=OP.add)
                gt = spool.tile([P, NCHUNK * EDIM], bf, tag="gt", name="gt")
                for c in range(NCHUNK):
                    tpack([grp[t][:, c * P:(c + 1) * P] for t in range(2)],
                          gt[:, c * EDIM:(c + 1) * EDIM])
                m_ps = sm_ps.tile([P, 512], f32, tag="sps", name="sps")
                for c in range(NCHUNK):
                    r0 = W2 * c if c < 4 else 0
                    dst = (m_ps[r0:r0 + W2, 0:EDIM] if c < 4
                           else m_ps[0:W2, EDIM:2 * EDIM])
                    nc.tensor.matmul(dst, lhsT=s2_sb[:, c, :],
                                     rhs=gt[:, c * EDIM:(c + 1) * EDIM],
                                     start=True, stop=True,
                                     skip_group_check=True,
                                     tile_position=(0, r0))
                return m_ps

            def m_to_sbuf(m_ps):
                mT = spool.tile([P, 512], f32, tag="mT", name="mT")
                nc.scalar.activation(mT[:, 0:EDIM], m_ps[:, 0:EDIM], AF.Copy)
                nc.scalar.activation(mT[0:W2, EDIM:2 * EDIM],
                                     m_ps[0:W2, EDIM:2 * EDIM], AF.Copy)
                return mT

            def m_dimmajor(mT_sb):
                """m^T sbuf -> mdm bf16 [P, 2(dt), B_S] (dim-major m)."""
                ps = sm_ps.tile([P, 512], f32, tag="sps", name="sps")
                for t in range(2):
                    nc.tensor.transpose(ps[:, t * B_S:t * B_S + P],
                                        mT_sb[:, t * P:(t + 1) * P],
                                        ident[:])
                    nc.tensor.transpose(
                        ps[:, t * B_S + P:t * B_S + B_S],
                        mT_sb[0:W2, EDIM + t * P:EDIM + (t + 1) * P],
                        ident[0:W2, 0:W2])
                out = spool.tile([P, 2 * B_S], bf, tag="mdm", name="mdm")
                nc.scalar.activation(out[:], ps[:, 0:2 * B_S], AF.Copy)
                return out

            # ---------------- layers 0, 1
            for i in range(2):
                x2 = [xpool.tile([P, A_S], bf, tag=f"x2_{t}", name=f"x2_{t}")
                      for t in range(2)]
                for t in range(2):
                    for j in range(NBG):
                        js_ = slice(j * BG_AGENTS, (j + 1) * BG_AGENTS)
                        nc.vector.tensor_tensor(out=x2[t][:, js_],
                                                in0=emb[i][t][:, js_],
                                                in1=recipb[:, js_],
                                                op=OP.mult)
                m_ps = segsum(emb[i])
                mT_sb_dbg = m_to_sbuf(m_ps)
                if _DEBUG and s == 0 and i == 0:
                    for t in range(2):
                        nc.sync.dma_start(dbg_x2[t], x2[t][:])
                    nc.sync.dma_start(dbg_mT[:], mT_sb_dbg[:])
                mdm = m_dimmajor(mT_sb_dbg)
                r_ps = sm_ps.tile([P, 512], f32, tag="sps", name="sps")
                for dt in range(2):
                    sl = r_ps[:, dt * B_S:(dt + 1) * B_S]
                    for kt in range(2):
                        nc.tensor.matmul(
                            sl, lhsT=wrT[:, i, kt, dt, :],
                            rhs=mdm[:, kt * B_S:(kt + 1) * B_S],
                            start=(kt == 0), stop=False)
                    nc.tensor.matmul(sl, lhsT=bias_sb[:, i, dt, :],
                                     rhs=lenm1[:], start=False, stop=True)
                r_sb = spool.tile([P, 2 * B_S], f32, tag="r_sb", name="r_sb")
                nc.scalar.activation(r_sb[:], r_ps[:, 0:2 * B_S], AF.Copy)
                if _DEBUG and s == 0 and i == 0:
                    nc.sync.dma_start(dbg_r[:], r_sb[:])
                # R^T at alignments 0 / 64 / 128 -> rt [P, 3, 256] bf16
                rt = spool.tile([P, 3, EDIM], bf, tag="rt", name="rt")
                nc.vector.memset(rt[:], 0.0)
                rt_ps = sm_ps.tile([P, 512], f32, tag="sps", name="sps")
                for dt in range(2):
                    nc.tensor.transpose(
                        rt_ps[:, dt * P:(dt + 1) * P],
                        r_sb[:, dt * B_S:dt * B_S + P], ident[:])
                nc.scalar.activation(rt[:, 0, :], rt_ps[:, 0:EDIM], AF.Copy)
                rt_ps2 = sm_ps.tile([P, 512], f32, tag="sps", name="sps")
                for dt in range(2):
                    nc.tensor.transpose(
                        rt_ps2[0:96, dt * P:(dt + 1) * P],
                        r_sb[:, dt * B_S + 64:dt * B_S + B_S], ident[:])
                    nc.tensor.transpose(
                        rt_ps2[0:W2, EDIM + dt * P:EDIM + dt * P + P],
                        r_sb[:, dt * B_S + P:dt * B_S + B_S], ident[:])
                nc.scalar.activation(rt[0:96, 1, :], rt_ps2[0:96, 0:EDIM],
                                     AF.Copy)
                nc.scalar.activation(rt[0:W2, 2, :],
                                     rt_ps2[0:W2, EDIM:2 * EDIM], AF.Copy)
                # main matmuls: weights-outer over 4-block chunks so
                # consecutive matmuls share the stationary operand
                for j0 in range(0, NBG, 4):
                    jn = min(4, NBG - j0)
                    jss = [slice((j0 + jj) * BG_AGENTS,
                                 (j0 + jj + 1) * BG_AGENTS)
                           for jj in range(jn)]
                    al = min(j0 // 4, 2)
                    for dt in range(2):
                        pss = [main_ps.tile([P, BG_AGENTS], f32, tag="main",
                                            name="main")
                               for _ in range(jn)]
                        for kt in range(2):
                            for jj in range(jn):
                                nc.tensor.matmul(
                                    pss[jj][:], lhsT=wlT[:, i, kt, dt, :],
                                    rhs=emb[i][kt][:, jss[jj]],
                                    start=(kt == 0), stop=False)
                        for kt in range(2):
                            for jj in range(jn):
                                nc.tensor.matmul(
                                    pss[jj][:], lhsT=wrTn[:, i, kt, dt, :],
                                    rhs=x2[kt][:, jss[jj]],
                                    start=False, stop=False)
                        for jj in range(jn):
                            nc.tensor.matmul(
                                pss[jj][:],
                                lhsT=rt[:, al, dt * P:(dt + 1) * P],
                                rhs=xsel[:, jss[jj]],
                                start=False, stop=True)
                        for jj in range(jn):
                            nc.scalar.activation(emb[i + 1][dt][:, jss[jj]],
                                                 pss[jj][:], AF.Relu)
                if _DEBUG and s == 0 and i == 0:
                    for t in range(2):
                        nc.sync.dma_start(dbg_emb1[t], emb[1][t][:])

            # ---------------- final segsum + head + log_softmax
            m_ps = segsum(emb[2])
            mdm = m_dimmajor(m_to_sbuf(m_ps))
            h_ps = sm_ps.tile([P, 512], f32, tag="sps", name="sps")
            hsl = h_ps[0:N_ACTIONS, 0:B_S]
            for kt in range(2):
                nc.tensor.matmul(hsl, lhsT=whT[:, kt, :],
                                 rhs=mdm[:, kt * B_S:(kt + 1) * B_S],
                                 start=(kt == 0), stop=False)
            nc.tensor.matmul(hsl, lhsT=bh_sb[:], rhs=ones_sb[:],
                             start=False, stop=True)
            h_sb = spool.tile([N_ACTIONS, B_S], f32, tag="h_sb", name="h_sb")
            nc.scalar.activation(h_sb[:], hsl, AF.Copy)
            if _DEBUG and s == 0:
                nc.sync.dma_start(dbg_h[:], h_sb[:])
            lg_ps = sm_ps.tile([P, 512], f32, tag="sps", name="sps")
            nc.tensor.transpose(lg_ps[:, 0:N_ACTIONS], h_sb[:, 0:P],
                                ident[0:N_ACTIONS, 0:N_ACTIONS])
            nc.tensor.transpose(lg_ps[0:W2, N_ACTIONS:2 * N_ACTIONS],
                                h_sb[:, P:B_S],
                                ident[0:N_ACTIONS, 0:N_ACTIONS])
            lg = spool.tile([P, 2 * N_ACTIONS], f32, tag="lg_sb", name="lg_sb")
            nc.scalar.activation(lg[:, 0:N_ACTIONS], lg_ps[:, 0:N_ACTIONS],
                                 AF.Copy)
            nc.scalar.activation(lg[0:W2, N_ACTIONS:2 * N_ACTIONS],
                                 lg_ps[0:W2, N_ACTIONS:2 * N_ACTIONS],
                                 AF.Copy)
            for part in range(2):
                rows = P if part == 0 else B_S - P
                src = lg[0:rows, part * N_ACTIONS:(part + 1) * N_ACTIONS]
                mx = spool.tile([P, 1], f32, tag="mx", name="mx")
                nc.vector.tensor_reduce(mx[0:rows, :], src,
                                        axis=mybir.AxisListType.X,
                                        op=OP.max)
                shv = spool.tile([P, N_ACTIONS], f32, tag="shift", name="shift")
                nc.vector.tensor_tensor(
                    out=shv[0:rows, :], in0=src,
                    in1=mx[0:rows, :].to_broadcast([rows, N_ACTIONS]),
                    op=OP.subtract)
                ex = spool.tile([P, N_ACTIONS], f32, tag="ex", name="ex")
                se = spool.tile([P, 1], f32, tag="se", name="se")
                nc.scalar.activation(ex[0:rows, :], shv[0:rows, :], AF.Exp,
                                     accum_out=se[0:rows, :])
                lse = spool.tile([P, 1], f32, tag="lse", name="lse")
                nc.scalar.activation(lse[0:rows, :], se[0:rows, :], AF.Ln)
                res = spool.tile([P, N_ACTIONS], f32, tag="res", name="res")
                nc.vector.tensor_tensor(
                    out=res[0:rows, :], in0=shv[0:rows, :],
                    in1=lse[0:rows, :].to_broadcast([rows, N_ACTIONS]),
                    op=OP.subtract)
                nc.sync.dma_start(out_d[s, part * P:part * P + rows, :],
                                  res[0:rows, :])
    nc.compile()
    return nc


# ================================================================== kernel
def kernel(**inputs):
    per_core, out_map, _ = _build_host_inputs(
        inputs["x"], inputs["batch_idx"], inputs["batch_len"],
        inputs["emb_table"], inputs["W0"], inputs["b0"], inputs["W1"],
        inputs["b1"], inputs["Wh"], inputs["bh"])

    if "prog" not in _PROGRAM_CACHE:
        _PROGRAM_CACHE["prog"] = _build_program()
    nc = _PROGRAM_CACHE["prog"]

    from concourse.bass_utils import run_bass_kernel_spmd
    res = run_bass_kernel_spmd(nc, per_core, core_ids=list(range(N_CORES)))

    logp = np.zeros((N_BATCHES, N_ACTIONS), np.float32)
    for core in range(N_CORES):
        out = np.asarray(res.results[core]["out"], np.float32)
        for s in range(NSUB):
            sh = core * NSUB + s
            v = out_map[sh] >= 0
            logp[out_map[sh][v]] = out[s][v]
    return (logp,)



# revision 22
# speedup vs baseline: 1.4727x; 1.0255x over previous
"""Trainium2 Bass kernel for nn_Commnet (gnn_message_passing).

kernel(**inputs) takes FULL unsharded numpy inputs, returns (logp [4096,16],)
computed across 8 NeuronCores (SPMD single program; per-core structure is
carried entirely in input tensors).

Sharding: batches bin-packed into 32 sub-shards (4/core); each sub-shard =
10 batch-groups of 16 batch slots padded to exactly 512 agent slots, so every
512-agent matmul block has a static 16-batch selector window -> the program
is identical on all cores.

EmbeddingBag(mean): table cast to bf16 pre-scaled by 1/8 on host (exact);
the host pre-permutes the per-agent word rows into dim-major slabs
slabT[t][dim, agent*8+word] (a pure row gather/layout, no arithmetic) so the
device streams them with large contiguous HWDGE DMAs at line rate; the
bag-sum is a DVE group-reduce(8) along the free dim, directly producing the
dim-major agent embeddings (no PE transposes, no SWDGE descriptor storm).

Middle layers: emb' = relu(W_l@emb - W_r@(emb*recip) + R'@sel) where
R' = W_r@m + b (x) (len-0.99999) folds expansion+bias; sel is a banded
one-hot*recip selector (1 extra k-tile per block). m = segment sum via DVE
group-sum(8) -> PE transpose -> banded matmuls into disjoint PSUM windows.
Padded agents stay exactly 0 through all layers.
"""

from contextlib import ExitStack

import numpy as np
import ml_dtypes

import concourse.bass as bass
import concourse.bacc as bacc
import concourse.tile as tile
from concourse import mybir
from concourse.masks import make_identity

N_WORDS = 32000
EDIM = 256
N_AGENTS = 131072
BAG = 8
N_BATCHES = 4096
N_ACTIONS = 16
NLAYERS = 3

N_CORES = 8
NSUB = 4                  # sub-shards per core
NSHARD = N_CORES * NSUB   # 32
NBG = 10                  # batch-groups per sub-shard
BG_BATCHES = 16
BG_AGENTS = 512
B_S = NBG * BG_BATCHES    # 160
A_S = NBG * BG_AGENTS     # 5120
G = 8
NG = A_S // G             # 640
NCHUNK = NG // 128        # 5
W2 = 2 * BG_BATCHES       # 32
P = 128
ZPAD_ROW = N_WORDS
DT = mybir.dt
AF = mybir.ActivationFunctionType
OP = mybir.AluOpType

_PROGRAM_CACHE = {}
_DEBUG = False


# ================================================================ host prep
def _pack_batches(counts):
    padded = ((counts + G - 1) // G) * G
    ngroups = NSHARD * NBG
    cap = np.full(ngroups, BG_AGENTS, dtype=np.int64)
    slots = np.full(ngroups, BG_BATCHES, dtype=np.int64)
    members = [[] for _ in range(ngroups)]
    for b in np.argsort(-padded, kind="stable"):
        ok = (cap >= padded[b]) & (slots > 0)
        if not ok.any():
            raise RuntimeError("bin packing failed")
        g = int(np.argmax(np.where(ok, cap, -1)))
        members[g].append(int(b))
        cap[g] -= padded[b]
        slots[g] -= 1
    return members, padded


def _build_host_inputs(x, batch_idx, batch_len, emb_table, W0, b0, W1, b1,
                       Wh, bh):
    bf16 = ml_dtypes.bfloat16
    x = np.asarray(x, dtype=np.int64)
    batch_idx = np.asarray(batch_idx, dtype=np.int64)
    batch_len64 = np.asarray(batch_len, dtype=np.float64)

    counts = np.bincount(batch_idx, minlength=N_BATCHES).astype(np.int64)
    starts = np.concatenate([[0], np.cumsum(counts)[:-1]])
    members, padded = _pack_batches(counts)

    # table rows pre-scaled by 1/8 (exact), bf16, with a zero pad row;
    # uint16 view for fast host-side take/transpose.
    tblu = np.concatenate(
        [np.asarray(emb_table, np.float32) / 8.0,
         np.zeros((1, EDIM), np.float32)], 0).astype(bf16).view(np.uint16)
    # dim-major word-plane slabs: slab_all[sh, t, k, p, a] =
    # table[x[a*BAG+k], t*128+p] / 8 (zeros for padded agent slots).
    # The device bag-sums the 8 k-planes via SWDGE accumulate-DMA (CCE ADD).
    slab_all = np.empty((NSHARD, 2, BAG, P, A_S), dtype=np.uint16)
    xsel_all = np.zeros((NSHARD, P, A_S), dtype=bf16)
    recip_all = np.zeros((NSHARD, P, A_S), dtype=bf16)
    s2_all = np.zeros((NSHARD, P, NCHUNK, W2), dtype=bf16)
    lenm1_all = np.zeros((NSHARD, 1, B_S), dtype=bf16)
    gcnt_all = np.full((NSHARD, 1, NBG // 2), BG_AGENTS * 2, dtype=np.int32)
    out_map = np.full((NSHARD, B_S), -1, dtype=np.int64)
    dbg_slots = []
    recip_f = (1.0 / (batch_len64 - 0.99999)).astype(np.float32)

    for sh in range(NSHARD):
        idx_flat = np.full((BAG, A_S), ZPAD_ROW, dtype=np.int64)
        a_of_slot = np.full(A_S, -1, dtype=np.int64)
        b_of_slot = np.full(A_S, -1, dtype=np.int64)
        grp_content = np.zeros(NBG, np.int64)
        for bg in range(NBG):
            pos = bg * BG_AGENTS
            for sl, b in enumerate(members[sh * NBG + bg]):
                lb = bg * BG_BATCHES + sl
                out_map[sh, lb] = b
                lenm1_all[sh, 0, lb] = np.float32(batch_len64[b] - 0.99999)
                n = int(counts[b])
                a_of_slot[pos:pos + n] = np.arange(starts[b], starts[b] + n)
                b_of_slot[pos:pos + n] = lb
                pos += int(padded[b])
            grp_content[bg] = pos - bg * BG_AGENTS
        slots = np.nonzero(a_of_slot >= 0)[0]
        ags = a_of_slot[slots]
        for k in range(BAG):
            idx_flat[k, slots] = x[ags * BAG + k]

        # pre-gather + transpose to dim-major word planes
        for k in range(BAG):
            gath = tblu[idx_flat[k]]               # [A_S, 256]
            slab_all[sh, :, k] = gath.T.reshape(2, P, A_S)

        lb_real = b_of_slot[slots]
        rec = recip_f[out_map[sh, lb_real]]
        recip_row = np.zeros(A_S, np.float32)
        recip_row[slots] = rec
        recip_all[sh] = np.broadcast_to(recip_row.astype(bf16), (P, A_S))

        j_of_slot = slots // BG_AGENTS
        w0_al = np.where(j_of_slot >= 8, 128, (j_of_slot // 4) * 64)
        r = lb_real - w0_al
        assert (r >= 0).all() and (r < P).all()
        xs = np.zeros((P, A_S), np.float32)
        xs[r, slots] = rec
        xsel_all[sh] = xs.astype(bf16)

        dbg_slots.append((a_of_slot.copy(), b_of_slot.copy()))
        g_b = b_of_slot[::G]
        for c in range(NCHUNK):
            gl = np.arange(P)
            gb = g_b[c * P + gl]
            v = gb >= 0
            w = gb[v] - W2 * c
            assert (w >= 0).all() and (w < W2).all()
            s2_all[sh][gl[v], c, w] = 1.0

    W0 = np.asarray(W0, np.float32)
    W1 = np.asarray(W1, np.float32)
    wl = np.stack([W0[:, :EDIM].T, W1[:, :EDIM].T])   # [layer, 256k, 256d]
    wr = np.stack([W0[:, EDIM:].T, W1[:, EDIM:].T])

    def tiles(w):  # [2,256,256] -> [128, 2(layer), 2(kt), 2(dt), 128]
        t = w.reshape(2, 2, P, 2, P).transpose(2, 0, 1, 3, 4)
        return np.ascontiguousarray(t).astype(bf16)

    host = {
        "wlT": tiles(wl),
        "wrTn": tiles(-wr),
        "wrT": tiles(wr),
        "bias": np.ascontiguousarray(
            np.stack([np.asarray(b0, np.float32), np.asarray(b1, np.float32)])
            .reshape(2, 2, P).transpose(1, 0, 2)[None]  # wrong axis order?
        ),
        "whT": np.ascontiguousarray(
            np.asarray(Wh, np.float32).T.reshape(2, P, N_ACTIONS)
            .transpose(1, 0, 2)).astype(bf16),
        "bh": np.asarray(bh, np.float32).reshape(1, N_ACTIONS).astype(bf16),
        "ones_b": np.ones((1, B_S), bf16),
    }
    # bias layout: [1, 2(layer), 2(dt), 128]
    bias = np.stack([np.asarray(b0, np.float32),
                     np.asarray(b1, np.float32)]).reshape(2, 2, P)
    host["bias"] = bias[None].astype(bf16)

    slab_bf = slab_all.view(bf16)
    per_core = []
    for core in range(N_CORES):
        s0 = core * NSUB
        m = dict(host)
        m["slabT"] = slab_bf[s0:s0 + NSUB]
        m["xsel"] = xsel_all[s0:s0 + NSUB]
        m["recipb"] = recip_all[s0:s0 + NSUB]
        m["s2"] = s2_all[s0:s0 + NSUB]
        m["lenm1"] = lenm1_all[s0:s0 + NSUB]
        per_core.append(m)
    return per_core, out_map, dbg_slots


# ============================================================ device program
def _build_program():
    nc = bacc.Bacc("TRN2")
    bf, f32, i32 = DT.bfloat16, DT.float32, DT.int32

    slab_d = nc.dram_tensor("slabT", [NSUB, 2, BAG, P, A_S], bf,
                            kind="ExternalInput")
    xsel_d = nc.dram_tensor("xsel", [NSUB, P, A_S], bf, kind="ExternalInput")
    recip_d = nc.dram_tensor("recipb", [NSUB, P, A_S], bf,
                             kind="ExternalInput")
    s2_d = nc.dram_tensor("s2", [NSUB, P, NCHUNK, W2], bf,
                          kind="ExternalInput")
    lenm1_d = nc.dram_tensor("lenm1", [NSUB, 1, B_S], bf,
                             kind="ExternalInput")
    wlT_d = nc.dram_tensor("wlT", [P, 2, 2, 2, P], bf, kind="ExternalInput")
    wrTn_d = nc.dram_tensor("wrTn", [P, 2, 2, 2, P], bf, kind="ExternalInput")
    wrT_d = nc.dram_tensor("wrT", [P, 2, 2, 2, P], bf, kind="ExternalInput")
    bias_d = nc.dram_tensor("bias", [1, 2, 2, P], bf, kind="ExternalInput")
    whT_d = nc.dram_tensor("whT", [P, 2, N_ACTIONS], bf,
                           kind="ExternalInput")
    bh_d = nc.dram_tensor("bh", [1, N_ACTIONS], bf, kind="ExternalInput")
    ones_d = nc.dram_tensor("ones_b", [1, B_S], bf, kind="ExternalInput")
    out_d = nc.dram_tensor("out", [NSUB, B_S, N_ACTIONS], f32,
                           kind="ExternalOutput")
    if _DEBUG:
        dbg_emb0 = nc.dram_tensor("dbg_emb0", [2, P, A_S], bf,
                                  kind="ExternalOutput")
        dbg_emb1 = nc.dram_tensor("dbg_emb1", [2, P, A_S], bf,
                                  kind="ExternalOutput")
        dbg_mT = nc.dram_tensor("dbg_mT", [P, 512], f32,
                                kind="ExternalOutput")
        dbg_x2 = nc.dram_tensor("dbg_x2", [2, P, A_S], bf,
                                kind="ExternalOutput")
        dbg_r = nc.dram_tensor("dbg_r", [P, 2 * B_S], f32,
                               kind="ExternalOutput")
        dbg_h = nc.dram_tensor("dbg_h", [N_ACTIONS, B_S], f32,
                               kind="ExternalOutput")

    with tile.TileContext(nc) as tc, ExitStack() as ctx:
        consts = ctx.enter_context(tc.tile_pool(name="consts", bufs=1))
        wpool = ctx.enter_context(tc.tile_pool(name="wpool", bufs=1))
        gpool = ctx.enter_context(tc.tile_pool(name="gath", bufs=2))
        epool = ctx.enter_context(tc.tile_pool(name="emb", bufs=2))
        e1pool = ctx.enter_context(tc.tile_pool(name="emb1", bufs=2))
        e2pool = ctx.enter_context(tc.tile_pool(name="emb2", bufs=1))
        xpool = ctx.enter_context(tc.tile_pool(name="x2p", bufs=1))
        spool = ctx.enter_context(tc.tile_pool(name="small", bufs=2))
        main_ps = ctx.enter_context(
            tc.tile_pool(name="mps", bufs=5, space="PSUM"))
        tp_ps = ctx.enter_context(
            tc.tile_pool(name="tps", bufs=1, space="PSUM"))
        sm_ps = ctx.enter_context(
            tc.tile_pool(name="sps", bufs=2, space="PSUM"))

        ident = consts.tile([P, P], f32, tag="ident", name="ident")
        make_identity(nc, ident[:])

        wlT = wpool.tile([P, 2, 2, 2, P], bf, tag="wlT", name="wlT")
        nc.sync.dma_start(wlT[:], wlT_d[:])
        wrTn = wpool.tile([P, 2, 2, 2, P], bf, tag="wrTn", name="wrTn")
        nc.sync.dma_start(wrTn[:], wrTn_d[:])
        wrT = wpool.tile([P, 2, 2, 2, P], bf, tag="wrT", name="wrT")
        nc.sync.dma_start(wrT[:], wrT_d[:])
        bias_sb = wpool.tile([1, 2, 2, P], bf, tag="bias", name="bias")
        nc.sync.dma_start(bias_sb[:], bias_d[:])
        whT = wpool.tile([P, 2, N_ACTIONS], bf, tag="whT", name="whT")
        nc.sync.dma_start(whT[:], whT_d[:])
        bh_sb = wpool.tile([1, N_ACTIONS], bf, tag="bh", name="bh")
        nc.sync.dma_start(bh_sb[:], bh_d[:])
        ones_sb = wpool.tile([1, B_S], bf, tag="ones", name="ones")
        nc.sync.dma_start(ones_sb[:], ones_d[:])

        def tpack(src_aps, dst_ap):
            """PE-transpose [p<=128, w<=128] fp32 APs into one psum bank,
            then one ACT copy (w/ cast) into dst_ap (columns concatenated).
            Each src must have 128 partitions."""
            ps = tp_ps.tile([P, 512], f32, tag="tpack", name="tpack")
            col = 0
            for a in src_aps:
                w = a.shape[-1]
                nc.tensor.transpose(ps[:, col:col + w], a, ident[:])
                col += w
            nc.scalar.activation(dst_ap, ps[:, :col], AF.Copy)

        for s in range(NSUB):
            # ---------------- Phase A: stream pre-gathered slabs + bag-sum
            xsel = gpool.tile([P, A_S], bf, tag="xsel", name="xsel", bufs=1)
            nc.sync.dma_start(xsel[:], xsel_d[s])
            recipb = gpool.tile([P, A_S], bf, tag="recipb", name="recipb", bufs=1)
            nc.sync.dma_start(recipb[:], recip_d[s])
            s2_sb = gpool.tile([P, NCHUNK, W2], bf, tag="s2", name="s2")
            nc.sync.dma_start(s2_sb[:], s2_d[s])
            lenm1 = gpool.tile([1, B_S], bf, tag="lenm1", name="lenm1")
            nc.sync.dma_start(lenm1[:], lenm1_d[s])

            emb = [None] * NLAYERS
            emb[0] = [epool.tile([P, A_S], bf, tag=f"emb0_{t}", name=f"emb0_{t}")
                      for t in range(2)]
            emb[1] = [e1pool.tile([P, A_S], bf, tag=f"emb1_{t}", name=f"emb1_{t}")
                      for t in range(2)]
            emb[2] = [e2pool.tile([P, A_S], bf, tag=f"emb2_{t}", name=f"emb2_{t}")
                      for t in range(2)]

            # bag-sum of the 8 word planes, split across units: plane 0
            # lands via plain HWDGE, plane 1 accumulates inside the SDMA
            # engines (CCE ADD; descriptors >2048 elements corrupt, so
            # 2048-wide chunks; same-address RMW needs sem-ordered ops),
            # planes 2-7 go through a DVE pairwise add tree (2x bf16 uop;
            # tensor_reduce only has the 1x uop).
            CW = A_S // 4
            for t in range(2):
                nc.sync.dma_start(emb[0][t][:], slab_d[s, t, 0])
                for c0 in range(0, A_S, 2048):
                    c1 = min(c0 + 2048, A_S)
                    nc.gpsimd.dma_start(emb[0][t][:, c0:c1],
                                        slab_d[s, t, 1, :, c0:c1],
                                        accum_op=OP.add)
                for c in range(4):
                    st = gpool.tile([P, 6, CW], bf, tag="stage",
                                    name="stage", bufs=2)
                    cs = slice(c * CW, (c + 1) * CW)
                    for k in range(6):
                        nc.sync.dma_start(st[:, k, :],
                                          slab_d[s, t, 2 + k, :, cs])
                    for (d, a, b) in ((0, 0, 1), (2, 2, 3), (4, 4, 5),
                                     (0, 0, 2), (0, 0, 4)):
                        nc.vector.tensor_tensor(out=st[:, d, :],
                                                in0=st[:, a, :],
                                                in1=st[:, b, :], op=OP.add)
                    nc.vector.tensor_tensor(out=emb[0][t][:, cs],
                                            in0=emb[0][t][:, cs],
                                            in1=st[:, 0, :], op=OP.add)

            if _DEBUG and s == 0:
                for t in range(2):
                    nc.sync.dma_start(dbg_emb0[t], emb[0][t][:])

            # ---------------- helpers
            def segsum(src):
                """src = [t0, t1] bf16 [P, A_S] -> m^T psum [P, 512] f32:
                cols 0:256 = batches 0:128, cols 256:512 = batches 128:160
                (on partitions 0:32)."""
                grp = [spool.tile([P, NG], f32, tag=f"grp{t}", name=f"grp{t}",
                                  bufs=1)
                       for t in range(2)]
                gpb = BG_AGENTS // G
                for t in range(2):
                    for j in range(NBG):
                        nc.vector.tensor_reduce(
                            grp[t][:, j * gpb:(j + 1) * gpb],
                            src[t][:, j * BG_AGENTS:(j + 1) * BG_AGENTS]
                            .rearrange("p (g e) -> p g e", e=G),
                            axis=mybir.AxisListType.X, op=OP.add)
                gt = spool.tile([P, NCHUNK * EDIM], bf, tag="gt", name="gt")
                for c in range(NCHUNK):
                    tpack([grp[t][:, c * P:(c + 1) * P] for t in range(2)],
                          gt[:, c * EDIM:(c + 1) * EDIM])
                m_ps = sm_ps.tile([P, 512], f32, tag="sps", name="sps")
                for c in range(NCHUNK):
                    r0 = W2 * c if c < 4 else 0
                    dst = (m_ps[r0:r0 + W2, 0:EDIM] if c < 4
                           else m_ps[0:W2, EDIM:2 * EDIM])
                    nc.tensor.matmul(dst, lhsT=s2_sb[:, c, :],
                                     rhs=gt[:, c * EDIM:(c + 1) * EDIM],
                                     start=True, stop=True,
                                     skip_group_check=True,
                                     tile_position=(0, r0))
                return m_ps

            def m_to_sbuf(m_ps):
                mT = spool.tile([P, 512], f32, tag="mT", name="mT")
                nc.scalar.activation(mT[:, 0:EDIM], m_ps[:, 0:EDIM], AF.Copy)
                nc.scalar.activation(mT[0:W2, EDIM:2 * EDIM],
                                     m_ps[0:W2, EDIM:2 * EDIM], AF.Copy)
                return mT

            def m_dimmajor(mT_sb):
                """m^T sbuf -> mdm bf16 [P, 2(dt), B_S] (dim-major m)."""
                ps = sm_ps.tile([P, 512], f32, tag="sps", name="sps")
                for t in range(2):
                    nc.tensor.transpose(ps[:, t * B_S:t * B_S + P],
                                        mT_sb[:, t * P:(t + 1) * P],
                                        ident[:])
                    nc.tensor.transpose(
                        ps[:, t * B_S + P:t * B_S + B_S],
                        mT_sb[0:W2, EDIM + t * P:EDIM + (t + 1) * P],
                        ident[0:W2, 0:W2])
                out = spool.tile([P, 2 * B_S], bf, tag="mdm", name="mdm")
                nc.scalar.activation(out[:], ps[:, 0:2 * B_S], AF.Copy)
                return out

            # ---------------- layers 0, 1
            for i in range(2):
                x2 = [xpool.tile([P, A_S], bf, tag=f"x2_{t}", name=f"x2_{t}")
                      for t in range(2)]
                for t in range(2):
                    for j in range(NBG):
                        js_ = slice(j * BG_AGENTS, (j + 1) * BG_AGENTS)
                        nc.vector.tensor_tensor(out=x2[t][:, js_],
                                                in0=emb[i][t][:, js_],
                                                in1=recipb[:, js_],
                                                op=OP.mult)
                m_ps = segsum(emb[i])
                mT_sb_dbg = m_to_sbuf(m_ps)
                if _DEBUG and s == 0 and i == 0:
                    for t in range(2):
                        nc.sync.dma_start(dbg_x2[t], x2[t][:])
                    nc.sync.dma_start(dbg_mT[:], mT_sb_dbg[:])
                mdm = m_dimmajor(mT_sb_dbg)
                r_ps = sm_ps.tile([P, 512], f32, tag="sps", name="sps")
                for dt in range(2):
                    sl = r_ps[:, dt * B_S:(dt + 1) * B_S]
                    for kt in range(2):
                        nc.tensor.matmul(
                            sl, lhsT=wrT[:, i, kt, dt, :],
                            rhs=mdm[:, kt * B_S:(kt + 1) * B_S],
                            start=(kt == 0), stop=False)
                    nc.tensor.matmul(sl, lhsT=bias_sb[:, i, dt, :],
                                     rhs=lenm1[:], start=False, stop=True)
                r_sb = spool.tile([P, 2 * B_S], f32, tag="r_sb", name="r_sb")
                nc.scalar.activation(r_sb[:], r_ps[:, 0:2 * B_S], AF.Copy)
                if _DEBUG and s == 0 and i == 0:
                    nc.sync.dma_start(dbg_r[:], r_sb[:])
                # R^T at alignments 0 / 64 / 128 -> rt [P, 3, 256] bf16
                rt = spool.tile([P, 3, EDIM], bf, tag="rt", name="rt")
                nc.vector.memset(rt[:], 0.0)
                rt_ps = sm_ps.tile([P, 512], f32, tag="sps", name="sps")
                for dt in range(2):
                    nc.tensor.transpose(
                        rt_ps[:, dt * P:(dt + 1) * P],
                        r_sb[:, dt * B_S:dt * B_S + P], ident[:])
                nc.scalar.activation(rt[:, 0, :], rt_ps[:, 0:EDIM], AF.Copy)
                rt_ps2 = sm_ps.tile([P, 512], f32, tag="sps", name="sps")
                for dt in range(2):
                    nc.tensor.transpose(
                        rt_ps2[0:96, dt * P:(dt + 1) * P],
                        r_sb[:, dt * B_S + 64:dt * B_S + B_S], ident[:])
                    nc.tensor.transpose(
                        rt_ps2[0:W2, EDIM + dt * P:EDIM + dt * P + P],
                        r_sb[:, dt * B_S + P:dt * B_S + B_S], ident[:])
                nc.scalar.activation(rt[0:96, 1, :], rt_ps2[0:96, 0:EDIM],
                                     AF.Copy)
                nc.scalar.activation(rt[0:W2, 2, :],
                                     rt_ps2[0:W2, EDIM:2 * EDIM], AF.Copy)
                # main matmuls: weights-outer over 4-block chunks so
                # consecutive matmuls share the stationary operand
                for j0 in range(0, NBG, 4):
                    jn = min(4, NBG - j0)
                    jss = [slice((j0 + jj) * BG_AGENTS,
                                 (j0 + jj + 1) * BG_AGENTS)
                           for jj in range(jn)]
                    al = min(j0 // 4, 2)
                    for dt in range(2):
                        pss = [main_ps.tile([P, BG_AGENTS], f32, tag="main",
                                            name="main")
                               for _ in range(jn)]
                        for kt in range(2):
                            for jj in range(jn):
                                nc.tensor.matmul(
                                    pss[jj][:], lhsT=wlT[:, i, kt, dt, :],
                                    rhs=emb[i][kt][:, jss[jj]],
                                    start=(kt == 0), stop=False)
                        for kt in range(2):
                            for jj in range(jn):
                                nc.tensor.matmul(
                                    pss[jj][:], lhsT=wrTn[:, i, kt, dt, :],
                                    rhs=x2[kt][:, jss[jj]],
                                    start=False, stop=False)
                        for jj in range(jn):
                            nc.tensor.matmul(
                                pss[jj][:],
                                lhsT=rt[:, al, dt * P:(dt + 1) * P],
                                rhs=xsel[:, jss[jj]],
                                start=False, stop=True)
                        for jj in range(jn):
                            nc.scalar.activation(emb[i + 1][dt][:, jss[jj]],
                                                 pss[jj][:], AF.Relu)
                if _DEBUG and s == 0 and i == 0:
                    for t in range(2):
                        nc.sync.dma_start(dbg_emb1[t], emb[1][t][:])

            # ---------------- final segsum + head + log_softmax
            m_ps = segsum(emb[2])
            mdm = m_dimmajor(m_to_sbuf(m_ps))
            h_ps = sm_ps.tile([P, 512], f32, tag="sps", name="sps")
            hsl = h_ps[0:N_ACTIONS, 0:B_S]
            for kt in range(2):
                nc.tensor.matmul(hsl, lhsT=whT[:, kt, :],
                                 rhs=mdm[:, kt * B_S:(kt + 1) * B_S],
                                 start=(kt == 0), stop=False)
            nc.tensor.matmul(hsl, lhsT=bh_sb[:], rhs=ones_sb[:],
                             start=False, stop=True)
            h_sb = spool.tile([N_ACTIONS, B_S], f32, tag="h_sb", name="h_sb")
            nc.scalar.activation(h_sb[:], hsl, AF.Copy)
            if _DEBUG and s == 0:
                nc.sync.dma_start(dbg_h[:], h_sb[:])
            lg_ps = sm_ps.tile([P, 512], f32, tag="sps", name="sps")
            nc.tensor.transpose(lg_ps[:, 0:N_ACTIONS], h_sb[:, 0:P],
                                ident[0:N_ACTIONS, 0:N_ACTIONS])
            nc.tensor.transpose(lg_ps[0:W2, N_ACTIONS:2 * N_ACTIONS],
                                h_sb[:, P:B_S],
                                ident[0:N_ACTIONS, 0:N_ACTIONS])
            lg = spool.tile([P, 2 * N_ACTIONS], f32, tag="lg_sb", name="lg_sb")
            nc.scalar.activation(lg[:, 0:N_ACTIONS], lg_ps[:, 0:N_ACTIONS],
                                 AF.Copy)
            nc.scalar.activation(lg[0:W2, N_ACTIONS:2 * N_ACTIONS],
                                 lg_ps[0:W2, N_ACTIONS:2 * N_ACTIONS],
                                 AF.Copy)
            for part in range(2):
                rows = P if part == 0 else B_S - P
                src = lg[0:rows, part * N_ACTIONS:(part + 1) * N_ACTIONS]
                mx = spool.tile([P, 1], f32, tag="mx", name="mx")
                nc.vector.tensor_reduce(mx[0:rows, :], src,
                                        axis=mybir.AxisListType.X,
                                        op=OP.max)
                shv = spool.tile([P, N_ACTIONS], f32, tag="shift", name="shift")
                nc.vector.tensor_tensor(
                    out=shv[0:rows, :], in0=src,
                    in1=mx[0:rows, :].to_broadcast([rows, N_ACTIONS]),
                    op=OP.subtract)
                ex = spool.tile([P, N_ACTIONS], f32, tag="ex", name="ex")
                se = spool.tile([P, 1], f32, tag="se", name="se")
                nc.scalar.activation(ex[0:rows, :], shv[0:rows, :], AF.Exp,
                                     accum_out=se[0:rows, :])
                lse = spool.tile([P, 1], f32, tag="lse", name="lse")
                nc.scalar.activation(lse[0:rows, :], se[0:rows, :], AF.Ln)
                res = spool.tile([P, N_ACTIONS], f32, tag="res", name="res")
                nc.vector.tensor_tensor(
                    out=res[0:rows, :], in0=shv[0:rows, :],
                    in1=lse[0:rows, :].to_broadcast([rows, N_ACTIONS]),
                    op=OP.subtract)
                nc.sync.dma_start(out_d[s, part * P:part * P + rows, :],
                                  res[0:rows, :])
    nc.compile()
    return nc


# ================================================================== kernel
def kernel(**inputs):
    per_core, out_map, _ = _build_host_inputs(
        inputs["x"], inputs["batch_idx"], inputs["batch_len"],
        inputs["emb_table"], inputs["W0"], inputs["b0"], inputs["W1"],
        inputs["b1"], inputs["Wh"], inputs["bh"])

    if "prog" not in _PROGRAM_CACHE:
        _PROGRAM_CACHE["prog"] = _build_program()
    nc = _PROGRAM_CACHE["prog"]

    from concourse.bass_utils import run_bass_kernel_spmd
    res = run_bass_kernel_spmd(nc, per_core, core_ids=list(range(N_CORES)))

    logp = np.zeros((N_BATCHES, N_ACTIONS), np.float32)
    for core in range(N_CORES):
        out = np.asarray(res.results[core]["out"], np.float32)
        for s in range(NSUB):
            sh = core * NSUB + s
            v = out_map[sh] >= 0
            logp[out_map[sh][v]] = out[s][v]
    return (logp,)

